# revision 14
# baseline (speedup 1.0000x reference)
"""MiniCPM3 MLA attention (B=1, S=2048, 40 heads) on 8 Trainium2 NeuronCores.

Sharding: tensor-parallel over heads (5 heads/core) for q_b/kv_b/attention;
data-parallel over sequence for the low-rank a-projections (S/8 rows each,
then AllGather); o_proj row-parallel via AllToAll of per-head attention
outputs so each core computes full output channels for its S/8 sequence rows.

Device layout convention: activations are kept feature-major ("transposed",
features on SBUF partitions) so every matmul contraction runs over the
partition axis without any on-device transposes.

All matmuls run in bf16 (1 PE cycle/row) with fp32 PSUM accumulation, except
tiny fp32r helper matmuls (partition-axis sums / broadcasts). Softmax uses
multiplicative causal masking after exp (no max subtraction; logits are O(5)
so exp cannot overflow) and a fused ones-column in the PV matmul to produce
denominators.
"""
import sys
sys.path.insert(0, "/opt/trn_rl_repo")
from contextlib import ExitStack

import numpy as np
import ml_dtypes

import concourse.bass as bass
import concourse.mybir as mybir
import concourse.tile as tile
from concourse import bacc

# ---- problem dims (hardcoded per spec) ----
HIDDEN = 2560
N_HEADS = 40
Q_LORA = 768
KV_LORA = 256
ROPE_D = 32
NOPE_D = 64
V_D = 64
QHD = NOPE_D + ROPE_D            # 96
ROPE_BASE = 10000.0
EPS = 1e-6
SCALE = QHD ** -0.5

N_CORES = 8
S = 2048
SH = S // N_CORES                # 256 sequence rows per core
HPC = N_HEADS // N_CORES         # 5 heads per core
QB = 512                         # query block
NQB = S // QB                    # 4
KT = 128                         # key tile
NKT = S // KT                    # 16

F32 = mybir.dt.float32
F32R = mybir.dt.float32r
BF16 = mybir.dt.bfloat16

KO_H = HIDDEN // 128             # 20 k-tiles over hidden
MQ = Q_LORA // 128               # 6 m-tiles over q_lora
MKV = KV_LORA // 128             # 2 m-tiles over kv_lora
AGROW = Q_LORA + KV_LORA + ROPE_D  # 1056 rows in the all-gather payload


def r(ap):
    """bitcast an fp32 AP to fp32r for full-rate PE matmul."""
    return ap.bitcast(F32R)


def build_nc():
    nc = bacc.Bacc(trn_type="TRN2", target_bir_lowering=False, debug=False,
                   num_devices=N_CORES)

    # ---- per-core external inputs ----
    hT = nc.dram_tensor("hT", [HIDDEN, SH], BF16, kind="ExternalInput")
    wqa = nc.dram_tensor("wqa", [HIDDEN, Q_LORA], BF16, kind="ExternalInput")
    wkva = nc.dram_tensor("wkva", [HIDDEN, KV_LORA + 2 * ROPE_D], BF16, kind="ExternalInput")
    qlnw = nc.dram_tensor("qlnw", [Q_LORA, 1], F32, kind="ExternalInput")
    kvlnw = nc.dram_tensor("kvlnw", [KV_LORA, 1], F32, kind="ExternalInput")
    wqb = nc.dram_tensor("wqb", [Q_LORA, HPC * (QHD + ROPE_D)], BF16, kind="ExternalInput")
    wkvb = nc.dram_tensor("wkvb", [KV_LORA, HPC * (NOPE_D + V_D)], BF16, kind="ExternalInput")
    wo = nc.dram_tensor("wo", [N_HEADS * V_D, HIDDEN], BF16, kind="ExternalInput")
    cosS = nc.dram_tensor("cosS", [ROPE_D, SH], F32, kind="ExternalInput")
    sinS = nc.dram_tensor("sinS", [ROPE_D, SH], F32, kind="ExternalInput")
    cosF = nc.dram_tensor("cosF", [ROPE_D, S], F32, kind="ExternalInput")
    sinF = nc.dram_tensor("sinF", [ROPE_D, S], F32, kind="ExternalInput")
    maskT = nc.dram_tensor("maskT", [128, 4 * QB], BF16, kind="ExternalInput")

    # ---- per-core external output: o for this core's S-shard, [SH, HIDDEN] ----
    o_out = nc.dram_tensor("o_out", [SH, HIDDEN], F32, kind="ExternalOutput")

    Exp = mybir.ActivationFunctionType.Exp
    Sqrt = mybir.ActivationFunctionType.Sqrt
    mult = mybir.AluOpType.mult
    add = mybir.AluOpType.add

    with tile.TileContext(nc) as tc, ExitStack() as top:
        dram = top.enter_context(tc.tile_pool(name="dram", bufs=1, space="DRAM"))
        ps_big = top.enter_context(tc.tile_pool(name="ps_big", bufs=3, space="PSUM"))
        ps_mid = top.enter_context(tc.tile_pool(name="ps_mid", bufs=2, space="PSUM"))
        consts = top.enter_context(tc.tile_pool(name="consts", bufs=1))
        tmp = top.enter_context(tc.tile_pool(name="tmp", bufs=3))

        # ---- DRAM collective buffers ----
        KVROW = KV_LORA + ROPE_D  # 288
        ag_kv_in = dram.tile([KVROW, SH], BF16)
        ag_kv_out = dram.tile([N_CORES * KVROW, SH], BF16, addr_space="Shared")
        ag_qa_in = dram.tile([Q_LORA, SH], BF16)
        ag_qa_out = dram.tile([N_CORES * Q_LORA, SH], BF16, addr_space="Shared")
        H1, H2 = 3, 2            # A2A split: heads 0-2 fire early, 3-4 at end
        a2a_in1 = dram.tile([N_CORES * H1 * V_D, SH], BF16)
        a2a_out1 = dram.tile([N_CORES * H1 * V_D, SH], BF16)
        a2a_in2 = dram.tile([N_CORES * H2 * V_D, SH], BF16)
        a2a_out2 = dram.tile([N_CORES * H2 * V_D, SH], BF16)

        # ---- small constants ----
        ones128 = consts.tile([128, 1], BF16)
        nc.vector.memset(ones128[:], 1.0)
        eps_sb = consts.tile([1, 1], F32)
        nc.vector.memset(eps_sb[:], EPS)
        mask_sb = consts.tile([128, 4 * QB], BF16)
        nc.sync.dma_start(mask_sb[:], maskT.ap())
        cosF_sb = consts.tile([ROPE_D, S], F32)
        nc.sync.dma_start(cosF_sb[:], cosF.ap())
        sinF_sb = consts.tile([ROPE_D, S], F32)
        nc.sync.dma_start(sinF_sb[:], sinF.ap())
        qlnw_sb = consts.tile([128, MQ], F32)
        for m in range(MQ):
            nc.sync.dma_start(qlnw_sb[:, m:m + 1], qlnw.ap()[128 * m:128 * (m + 1), :])
        kvlnw_sb = consts.tile([128, MKV], F32)
        for m in range(MKV):
            nc.sync.dma_start(kvlnw_sb[:, m:m + 1], kvlnw.ap()[128 * m:128 * (m + 1), :])

        # ================= Phase A: a-projections on the S-shard =============
        with ExitStack() as phA:
            pa = phA.enter_context(tc.tile_pool(name="phA", bufs=1))
            pa_sq = phA.enter_context(tc.tile_pool(name="phA_sq", bufs=3))

            hT_sb = pa.tile([128, KO_H * SH], BF16)
            for ko in range(KO_H):
                nc.sync.dma_start(hT_sb[:, SH * ko:SH * (ko + 1)],
                                  hT.ap()[128 * ko:128 * (ko + 1), :])
            wqa_sb = pa.tile([128, KO_H * Q_LORA], BF16)
            for ko in range(KO_H):
                nc.sync.dma_start(wqa_sb[:, Q_LORA * ko:Q_LORA * (ko + 1)],
                                  wqa.ap()[128 * ko:128 * (ko + 1), :])
            wkva_sb = pa.tile([128, KO_H * (KV_LORA + 2 * ROPE_D)], BF16)
            for ko in range(KO_H):
                nc.sync.dma_start(wkva_sb[:, 320 * ko:320 * (ko + 1)],
                                  wkva.ap()[128 * ko:128 * (ko + 1), :])

            def aproj_norm(n_m, w_sb, wwidth, moff, lnw_sb, fan_in, dst, dst_row):
                """matmul (feature-major) + rmsnorm over features; writes bf16
                normalized output into dst rows [dst_row, dst_row+128*n_m)."""
                x_sb = pa.tile([128, n_m * SH], F32, name=f"x_sb_{fan_in}")
                ssum = ps_mid.tile([128, 512], F32, name=f"ssum_{fan_in}", tag="ps")
                for m in range(n_m):
                    x_ps = ps_mid.tile([128, 512], F32, name=f"xps_{fan_in}_{m}", tag="ps")
                    for ko in range(KO_H):
                        nc.tensor.matmul(
                            x_ps[:, :SH],
                            lhsT=w_sb[:, wwidth * ko + moff + 128 * m:
                                      wwidth * ko + moff + 128 * (m + 1)],
                            rhs=hT_sb[:, SH * ko:SH * (ko + 1)],
                            start=(ko == 0), stop=(ko == KO_H - 1))
                    xs = x_sb[:, SH * m:SH * (m + 1)]
                    nc.vector.tensor_copy(xs, x_ps[:, :SH])
                    sq = pa_sq.tile([128, SH], BF16, name="sq")
                    nc.vector.tensor_mul(sq[:], xs, xs)
                    nc.tensor.matmul(ssum[:1, :SH], lhsT=ones128[:], rhs=sq[:],
                                     start=(m == 0), stop=(m == n_m - 1))
                # rnorm = 1/sqrt(mean + eps), broadcast to all partitions
                srt = pa_sq.tile([1, SH], F32, name="srt")
                nc.scalar.activation(srt[:], ssum[:1, :SH], Sqrt,
                                     scale=1.0 / fan_in, bias=eps_sb[:])
                rcp = pa_sq.tile([1, SH], F32, name="rcp")
                nc.vector.reciprocal_approx_fast(rcp[:], srt[:])
                rbc = pa_sq.tile([128, SH], F32, name="rbc")
                nc.gpsimd.partition_broadcast(rbc[:], rcp[:])
                for m in range(n_m):
                    outm = pa_sq.tile([128, SH], BF16, name="outm")
                    nc.vector.scalar_tensor_tensor(
                        outm[:], x_sb[:, SH * m:SH * (m + 1)],
                        lnw_sb[:, m:m + 1], rbc[:], op0=mult, op1=mult)
                    nc.sync.dma_start(
                        dst[dst_row + 128 * m:dst_row + 128 * (m + 1), :], outm[:])

            # --- kv path first so its (small) AllGather can overlap qa work ---
            aproj_norm(MKV, wkva_sb, KV_LORA + 2 * ROPE_D, 0, kvlnw_sb, KV_LORA,
                       ag_kv_in, 0)

            # k_pe: cols 256:288 = pe, 288:320 = pre-rotated pe; rope, no norm
            kpe_ps = ps_mid.tile([128, 512], F32, tag="ps")
            for ko in range(KO_H):
                nc.tensor.matmul(
                    kpe_ps[:2 * ROPE_D, :SH],
                    lhsT=wkva_sb[:, 320 * ko + 256:320 * ko + 320],
                    rhs=hT_sb[:, SH * ko:SH * (ko + 1)],
                    start=(ko == 0), stop=(ko == KO_H - 1))
            cosS_sb = pa.tile([ROPE_D, SH], F32)
            nc.sync.dma_start(cosS_sb[:], cosS.ap())
            sinS_sb = pa.tile([ROPE_D, SH], F32)
            nc.sync.dma_start(sinS_sb[:], sinS.ap())
            t1 = pa_sq.tile([ROPE_D, SH], F32, name="t1")
            nc.vector.tensor_mul(t1[:], kpe_ps[:ROPE_D, :SH], cosS_sb[:])
            t2 = pa_sq.tile([ROPE_D, SH], F32, name="t2")
            nc.vector.tensor_mul(t2[:], kpe_ps[ROPE_D:2 * ROPE_D, :SH], sinS_sb[:])
            kpe_bf = pa_sq.tile([ROPE_D, SH], BF16, name="kpe_bf")
            nc.vector.tensor_add(kpe_bf[:], t1[:], t2[:])
            nc.sync.dma_start(ag_kv_in[KV_LORA:KVROW, :], kpe_bf[:])

            nc.gpsimd.collective_compute(
                "AllGather", mybir.AluOpType.bypass,
                replica_groups=[list(range(N_CORES))],
                ins=[ag_kv_in[:]], outs=[ag_kv_out[:]])

            aproj_norm(MQ, wqa_sb, Q_LORA, 0, qlnw_sb, Q_LORA, ag_qa_in, 0)

        nc.gpsimd.collective_compute(
            "AllGather", mybir.AluOpType.bypass,
            replica_groups=[list(range(N_CORES))],
            ins=[ag_qa_in[:]], outs=[ag_qa_out[:]])

        WO_PRE = 6
        wo_pre = top.enter_context(tc.tile_pool(name="wo_pre", bufs=1))
        wo_pre_tiles = []
        for ko in range(WO_PRE):
            wt = wo_pre.tile([128, HIDDEN], BF16, name=f"wop_{ko}")
            nc.sync.dma_start(wt[:], wo.ap()[128 * ko:128 * (ko + 1), :])
            wo_pre_tiles.append(wt)

        # ================= Phases B-D ========================================
        with ExitStack() as phB:
            pb = phB.enter_context(tc.tile_pool(name="phB", bufs=1))
            qh_pool = phB.enter_context(tc.tile_pool(name="qh", bufs=2))
            kf_pool = phB.enter_context(tc.tile_pool(name="kf", bufs=1))
            pT_pool = phB.enter_context(tc.tile_pool(name="pT", bufs=2))
            at_pool = phB.enter_context(tc.tile_pool(name="at", bufs=2))

            KVROW = KV_LORA + ROPE_D
            # -- kv-side: assemble gathered activations (available first) --
            kvN = pb.tile([128, MKV * S], BF16)
            for m in range(MKV):
                for c in range(N_CORES):
                    nc.sync.dma_start(
                        kvN[:, S * m + SH * c:S * m + SH * (c + 1)],
                        ag_kv_out[KVROW * c + 128 * m:KVROW * c + 128 * (m + 1), :])
            kpeT = pb.tile([ROPE_D, S], BF16)
            for c in range(N_CORES):
                nc.sync.dma_start(
                    kpeT[:, SH * c:SH * (c + 1)],
                    ag_kv_out[KVROW * c + KV_LORA:KVROW * (c + 1), :])

            # -- b-projection weights --
            wqb_sb = pb.tile([128, MQ * (HPC * 128)], BF16)
            for ko in range(MQ):
                nc.sync.dma_start(wqb_sb[:, 640 * ko:640 * (ko + 1)],
                                  wqb.ap()[128 * ko:128 * (ko + 1), :])
            wkvb_sb = pb.tile([128, MKV * (HPC * 128)], BF16)
            for ko in range(MKV):
                nc.sync.dma_start(wkvb_sb[:, 640 * ko:640 * (ko + 1)],
                                  wkvb.ap()[128 * ko:128 * (ko + 1), :])

            # -- v for all local heads, seq-major with a fused ones column --
            # layout: v_all[:, h*(16*65) + st*65 + 0:64] = v tile, col 64 = 1.0
            v_all = pb.tile([128, HPC * NKT * (V_D + 1)], BF16)
            v_view = v_all.rearrange("p (h st c) -> p h st c", h=HPC, st=NKT)
            nc.gpsimd.memset(v_view[:, :, :, V_D:V_D + 1], 1.0)
            for st in range(NKT):
                v_ps = ps_mid.tile([128, 512], F32, tag="ps")
                for ko in range(MKV):
                    rhs = wkvb_sb[:, 640 * ko:640 * (ko + 1)].rearrange(
                        "p (h d) -> p h d", d=128)[:, :, NOPE_D:NOPE_D + V_D]
                    nc.tensor.matmul(v_ps[:, :HPC * V_D],
                                     lhsT=kvN[:, S * ko + 128 * st:S * ko + 128 * (st + 1)],
                                     rhs=rhs,
                                     start=(ko == 0), stop=(ko == MKV - 1))
                for h in range(HPC):
                    nc.vector.tensor_copy(
                        v_all[:, h * (NKT * 65) + st * 65:h * (NKT * 65) + st * 65 + V_D],
                        v_ps[:, V_D * h:V_D * (h + 1)])

            # ---- k_full^T for all local heads (kv path only) ----
            kfs = []
            for h in range(HPC):
                kf = kf_pool.tile([QHD, S], BF16, name=f"kf{h}")
                for nb in range(NQB):
                    kn_ps = ps_mid.tile([128, 512], F32, name="kn_ps", tag="ps")
                    for ko in range(MKV):
                        nc.tensor.matmul(
                            kn_ps[:NOPE_D, :],
                            lhsT=wkvb_sb[:, 640 * ko + 128 * h:640 * ko + 128 * h + NOPE_D],
                            rhs=kvN[:, S * ko + QB * nb:S * ko + QB * (nb + 1)],
                            start=(ko == 0), stop=(ko == MKV - 1))
                    nc.any.tensor_copy(kf[:NOPE_D, QB * nb:QB * (nb + 1)],
                                       kn_ps[:NOPE_D, :])
                nc.vector.tensor_copy(kf[NOPE_D:QHD, :], kpeT[:])
                kfs.append(kf)

            # -- qa-side: assemble gathered activations --
            qaN = pb.tile([128, MQ * S], BF16)
            for m in range(MQ):
                for c in range(N_CORES):
                    nc.sync.dma_start(
                        qaN[:, S * m + SH * c:S * m + SH * (c + 1)],
                        ag_qa_out[Q_LORA * c + 128 * m:Q_LORA * c + 128 * (m + 1), :])

            for h in range(HPC):
                kf = kfs[h]
                # ---- q^T for head h: [96, S], rows 64:96 roped ----
                qh = qh_pool.tile([QHD, S], BF16, name="qh")
                for nb in range(NQB):
                    q_ps = ps_mid.tile([128, 512], F32, name="q_ps", tag="ps")
                    for ko in range(MQ):
                        nc.tensor.matmul(
                            q_ps[:],
                            lhsT=wqb_sb[:, 640 * ko + 128 * h:640 * ko + 128 * (h + 1)],
                            rhs=qaN[:, S * ko + QB * nb:S * ko + QB * (nb + 1)],
                            start=(ko == 0), stop=(ko == MQ - 1))
                    cs = slice(QB * nb, QB * (nb + 1))
                    nc.any.tensor_copy(qh[:NOPE_D, cs], q_ps[:NOPE_D, :])
                    t1 = at_pool.tile([ROPE_D, QB], F32, name="t1")
                    nc.vector.tensor_mul(t1[:], q_ps[NOPE_D:QHD, :], cosF_sb[:, cs])
                    t2 = at_pool.tile([ROPE_D, QB], F32, name="t2")
                    nc.vector.tensor_mul(t2[:], q_ps[QHD:QHD + ROPE_D, :], sinF_sb[:, cs])
                    nc.vector.tensor_add(qh[NOPE_D:QHD, cs], t1[:], t2[:])

                # ---- attention for head h ----
                for qb in range(NQB):
                    nk = 4 * (qb + 1)
                    pT = pT_pool.tile([128, NKT * QB], BF16, name="pT")
                    for g in range(nk // 2):
                        sc_ps = ps_big.tile([128, 1024], F32, name="sc_ps", tag="big")
                        for k2 in range(2):
                            kt = 2 * g + k2
                            nc.tensor.matmul(
                                sc_ps[:, 512 * k2:512 * (k2 + 1)],
                                lhsT=kf[:, KT * kt:KT * (kt + 1)],
                                rhs=qh[:, QB * qb:QB * (qb + 1)],
                                start=True, stop=True)
                        nc.scalar.activation(pT[:, 1024 * g:1024 * (g + 1)],
                                             sc_ps[:], Exp, scale=SCALE)
                    for i, kt in enumerate(range(4 * qb, 4 * qb + 4)):
                        nc.gpsimd.tensor_mul(pT[:, QB * kt:QB * (kt + 1)],
                                             pT[:, QB * kt:QB * (kt + 1)],
                                             mask_sb[:, QB * i:QB * (i + 1)])
                    pv_ps = ps_mid.tile([128, 512], F32, name="pv_ps", tag="ps")
                    for kt in range(nk):
                        nc.tensor.matmul(
                            pv_ps[:V_D + 1, :],
                            lhsT=v_all[:, h * (NKT * 65) + 65 * kt:
                                       h * (NKT * 65) + 65 * kt + 65],
                            rhs=pT[:, QB * kt:QB * (kt + 1)],
                            start=(kt == 0), stop=(kt == nk - 1))
                    dn_sb = at_pool.tile([1, QB], F32, name="dn_sb")
                    nc.vector.tensor_copy(dn_sb[:], pv_ps[V_D:V_D + 1, :])
                    rcd = at_pool.tile([1, QB], F32, name="rcd")
                    nc.vector.reciprocal_approx_fast(rcd[:], dn_sb[:])
                    bc_sb = at_pool.tile([V_D, QB], F32, name="bc_sb")
                    nc.gpsimd.partition_broadcast(bc_sb[:], rcd[:])
                    attn_sb = at_pool.tile([V_D, QB], BF16, name="attn_sb")
                    nc.vector.tensor_mul(attn_sb[:], pv_ps[:V_D, :], bc_sb[:])
                    for half in range(2):
                        j2 = 2 * qb + half
                        if h < H1:
                            rowbase = (H1 * V_D) * j2 + V_D * h
                            dst = a2a_in1
                        else:
                            rowbase = (H2 * V_D) * j2 + V_D * (h - H1)
                            dst = a2a_in2
                        nc.sync.dma_start(
                            dst[rowbase:rowbase + V_D, :],
                            attn_sb[:, SH * half:SH * (half + 1)])

                if h == H1 - 1:
                    nc.gpsimd.collective_compute(
                        "AllToAll", mybir.AluOpType.bypass,
                        replica_groups=[list(range(N_CORES))],
                        ins=[a2a_in1[:]], outs=[a2a_out1[:]])

        # ================= AllToAll (heads 3-4) =============================
        nc.gpsimd.collective_compute(
            "AllToAll", mybir.AluOpType.bypass,
            replica_groups=[list(range(N_CORES))],
            ins=[a2a_in2[:]], outs=[a2a_out2[:]])

        # ================= Phase E: o-projection ============================
        with ExitStack() as phE:
            pe = phE.enter_context(tc.tile_pool(name="phE", bufs=1))
            attn_T = pe.tile([128, KO_H * SH], BF16)
            SPLIT = N_CORES * H1 * V_D // 128  # 12 k-tiles from A2A_1
            for ko in range(KO_H):
                if ko < SPLIT:
                    srcb = a2a_out1[128 * ko:128 * (ko + 1), :]
                else:
                    srcb = a2a_out2[128 * (ko - SPLIT):128 * (ko - SPLIT + 1), :]
                nc.sync.dma_start(attn_T[:, SH * ko:SH * (ko + 1)], srcb)
            wo_tiles = list(wo_pre_tiles)
            for ko in range(WO_PRE, KO_H):
                wt = pe.tile([128, HIDDEN], BF16, name=f"wo_{ko}")
                nc.sync.dma_start(wt[:], wo.ap()[128 * ko:128 * (ko + 1), :])
                wo_tiles.append(wt)
            for st in range(SH // 128):
                for no in range(HIDDEN // 512):
                    o_ps = ps_mid.tile([128, 512], F32, name="o_ps", tag="ps")
                    for ko in range(KO_H):
                        nc.tensor.matmul(
                            o_ps[:],
                            lhsT=attn_T[:, SH * ko + 128 * st:SH * ko + 128 * (st + 1)],
                            rhs=wo_tiles[ko][:, 512 * no:512 * (no + 1)],
                            start=(ko == 0), stop=(ko == KO_H - 1))
                    o_sb = pe.tile([128, 512], F32, name="o_sb", bufs=3)
                    nc.any.tensor_copy(o_sb[:], o_ps[:])
                    nc.sync.dma_start(
                        o_out.ap()[128 * st:128 * (st + 1), 512 * no:512 * (no + 1)],
                        o_sb[:])
    nc.compile()
    return nc


# =========================== host side ======================================

def _host_inputs(hidden_states, position_ids, w_qa, q_a_ln_w, w_qb, w_kva,
                 kv_a_ln_w, w_kvb, w_o):
    bf = ml_dtypes.bfloat16
    x = np.asarray(hidden_states, np.float32)[0]            # [S, HIDDEN]
    hT_full = np.ascontiguousarray(x.T).astype(bf)          # [HIDDEN, S]

    # rope cache gathered by position_ids (host-side prep; identity for arange)
    inv_freq = 1.0 / (ROPE_BASE ** (np.arange(0, ROPE_D, 2, dtype=np.float32) / ROPE_D))
    t = np.arange(S, dtype=np.float32)
    freqs = np.outer(t, inv_freq)
    emb = np.concatenate([freqs, freqs], axis=-1)           # [S, 32]
    cos = np.cos(emb).astype(np.float32)
    sin = np.sin(emb).astype(np.float32)
    pid = np.asarray(position_ids).reshape(-1).astype(np.int64)
    cosT = np.ascontiguousarray(cos[pid].T)                 # [32, S] f32
    sinT = np.ascontiguousarray(sin[pid].T)

    # causal mask diagonal tiles: m[i][k, q] = 1 if 128*i + k <= q (within 512)
    k_idx = np.arange(128)
    q_idx = np.arange(QB)
    masks = [((128 * i + k_idx[:, None]) <= q_idx[None, :]) for i in range(4)]
    maskT = np.concatenate(masks, axis=1).astype(bf)        # [128, 2048]

    def rot_cols(w):
        # rotate_half folded into weights: rot[:, :16] = -w[:, 16:], rot[:, 16:] = w[:, :16]
        h = ROPE_D // 2
        return np.concatenate([-w[..., h:], w[..., :h]], axis=-1)

    w_qa = np.asarray(w_qa, np.float32).astype(bf)
    w_kva = np.asarray(w_kva, np.float32)
    w_kva = np.concatenate([w_kva, rot_cols(w_kva[:, KV_LORA:])], axis=1).astype(bf)
    w_qb4 = np.asarray(w_qb, np.float32).reshape(Q_LORA, N_HEADS, QHD)
    w_qb4 = np.concatenate([w_qb4, rot_cols(w_qb4[:, :, NOPE_D:])], axis=2)
    w_kvb4 = np.asarray(w_kvb, np.float32).reshape(KV_LORA, N_HEADS, NOPE_D + V_D)
    w_o = np.asarray(w_o, np.float32)
    # permute rows to the split-A2A feature order: (rank, heads 0-2) then
    # (rank, heads 3-4), 64 v-dims per head
    perm_heads = ([5 * i + hh for i in range(N_CORES) for hh in range(3)] +
                  [5 * i + 3 + hh for i in range(N_CORES) for hh in range(2)])
    w_o = np.ascontiguousarray(
        w_o.reshape(N_HEADS, V_D, HIDDEN)[perm_heads].reshape(N_HEADS * V_D, HIDDEN)
    ).astype(bf)
    qln = np.asarray(q_a_ln_w, np.float32).reshape(Q_LORA, 1)
    kvln = np.asarray(kv_a_ln_w, np.float32).reshape(KV_LORA, 1)

    in_maps = []
    for c in range(N_CORES):
        hs = slice(SH * c, SH * (c + 1))
        heads = slice(HPC * c, HPC * (c + 1))
        in_maps.append({
            "hT": np.ascontiguousarray(hT_full[:, hs]),
            "wqa": w_qa,
            "wkva": w_kva,
            "qlnw": qln,
            "kvlnw": kvln,
            "wqb": np.ascontiguousarray(
                w_qb4[:, heads, :].reshape(Q_LORA, HPC * 128)).astype(bf),
            "wkvb": np.ascontiguousarray(
                w_kvb4[:, heads, :].reshape(KV_LORA, HPC * 128)).astype(bf),
            "wo": w_o,
            "cosS": np.ascontiguousarray(cosT[:, hs]),
            "sinS": np.ascontiguousarray(sinT[:, hs]),
            "cosF": cosT,
            "sinF": sinT,
            "maskT": maskT,
        })
    return in_maps


_CACHE = {}


def _get_runner():
    if "runner" not in _CACHE:
        from concourse.bass_utils import run_bass_kernel_spmd  # noqa: F401
        nc = build_nc()
        _CACHE["nc"] = nc
        _CACHE["runner"] = None
    return _CACHE["nc"]


def kernel(**inputs) -> np.ndarray:
    from concourse.bass_utils import run_bass_kernel_spmd
    nc = _get_runner()
    in_maps = _host_inputs(**inputs)
    res = run_bass_kernel_spmd(nc, in_maps, core_ids=list(range(N_CORES)))
    out = np.concatenate([res.results[c]["o_out"] for c in range(N_CORES)], axis=0)
    return out.reshape(1, S, HIDDEN).astype(np.float32)


# revision 15
# speedup vs baseline: 1.5268x; 1.5268x over previous
"""MiniCPM3 MLA attention (B=1, S=2048, 40 heads) on 8 Trainium2 NeuronCores.

Sharding: tensor-parallel over heads (5 heads/core) for q_b/kv_b/attention;
data-parallel over sequence for the low-rank a-projections (S/8 rows each,
then AllGather); o_proj row-parallel via AllToAll of per-head attention
outputs so each core computes full output channels for its S/8 sequence rows.

Device layout convention: activations are kept feature-major ("transposed",
features on SBUF partitions) so every matmul contraction runs over the
partition axis without any on-device transposes.

All matmuls run in bf16 (1 PE cycle/row) with fp32 PSUM accumulation, except
tiny fp32r helper matmuls (partition-axis sums / broadcasts). Softmax uses
multiplicative causal masking after exp (no max subtraction; logits are O(5)
so exp cannot overflow) and a fused ones-column in the PV matmul to produce
denominators.
"""
import sys
sys.path.insert(0, "/opt/trn_rl_repo")
from contextlib import ExitStack

import numpy as np
import ml_dtypes

import concourse.bass as bass
import concourse.mybir as mybir
import concourse.tile as tile
from concourse import bacc

# ---- problem dims (hardcoded per spec) ----
HIDDEN = 2560
N_HEADS = 40
Q_LORA = 768
KV_LORA = 256
ROPE_D = 32
NOPE_D = 64
V_D = 64
QHD = NOPE_D + ROPE_D            # 96
ROPE_BASE = 10000.0
EPS = 1e-6
SCALE = QHD ** -0.5

N_CORES = 8
S = 2048
SH = S // N_CORES                # 256 sequence rows per core
HPC = N_HEADS // N_CORES         # 5 heads per core
QB = 512                         # query block
NQB = S // QB                    # 4
KT = 128                         # key tile
NKT = S // KT                    # 16

F32 = mybir.dt.float32
F32R = mybir.dt.float32r
BF16 = mybir.dt.bfloat16

KO_H = HIDDEN // 128             # 20 k-tiles over hidden
MQ = Q_LORA // 128               # 6 m-tiles over q_lora
MKV = KV_LORA // 128             # 2 m-tiles over kv_lora
AGROW = Q_LORA + KV_LORA + ROPE_D  # 1056 rows in the all-gather payload


def r(ap):
    """bitcast an fp32 AP to fp32r for full-rate PE matmul."""
    return ap.bitcast(F32R)


def build_nc():
    nc = bacc.Bacc(trn_type="TRN2", target_bir_lowering=False, debug=False,
                   num_devices=N_CORES)

    # ---- per-core external inputs ----
    hT = nc.dram_tensor("hT", [HIDDEN, SH], BF16, kind="ExternalInput")
    wqa = nc.dram_tensor("wqa", [HIDDEN, Q_LORA], BF16, kind="ExternalInput")
    wkva = nc.dram_tensor("wkva", [HIDDEN, KV_LORA + 2 * ROPE_D], BF16, kind="ExternalInput")
    qlnw = nc.dram_tensor("qlnw", [Q_LORA, 1], F32, kind="ExternalInput")
    kvlnw = nc.dram_tensor("kvlnw", [KV_LORA, 1], F32, kind="ExternalInput")
    wqb = nc.dram_tensor("wqb", [Q_LORA, HPC * (QHD + ROPE_D)], BF16, kind="ExternalInput")
    wkvb = nc.dram_tensor("wkvb", [KV_LORA, HPC * (NOPE_D + V_D)], BF16, kind="ExternalInput")
    wo = nc.dram_tensor("wo", [N_HEADS * V_D, HIDDEN], BF16, kind="ExternalInput")
    cosS = nc.dram_tensor("cosS", [ROPE_D, SH], F32, kind="ExternalInput")
    sinS = nc.dram_tensor("sinS", [ROPE_D, SH], F32, kind="ExternalInput")
    cosF = nc.dram_tensor("cosF", [ROPE_D, S], F32, kind="ExternalInput")
    sinF = nc.dram_tensor("sinF", [ROPE_D, S], F32, kind="ExternalInput")
    maskT = nc.dram_tensor("maskT", [128, 4 * QB], BF16, kind="ExternalInput")

    # ---- per-core external output: o for this core's S-shard, [SH, HIDDEN] ----
    o_out = nc.dram_tensor("o_out", [SH, HIDDEN], F32, kind="ExternalOutput")

    Exp = mybir.ActivationFunctionType.Exp
    Sqrt = mybir.ActivationFunctionType.Sqrt
    mult = mybir.AluOpType.mult
    add = mybir.AluOpType.add

    with tile.TileContext(nc) as tc, ExitStack() as top:
        dram = top.enter_context(tc.tile_pool(name="dram", bufs=1, space="DRAM"))
        ps_big = top.enter_context(tc.tile_pool(name="ps_big", bufs=3, space="PSUM"))
        ps_mid = top.enter_context(tc.tile_pool(name="ps_mid", bufs=2, space="PSUM"))
        consts = top.enter_context(tc.tile_pool(name="consts", bufs=1))
        tmp = top.enter_context(tc.tile_pool(name="tmp", bufs=3))

        # ---- DRAM collective buffers ----
        KVROW = KV_LORA + ROPE_D  # 288
        ag_kv_in = dram.tile([KVROW, SH], BF16)
        ag_kv_out = dram.tile([N_CORES * KVROW, SH], BF16, addr_space="Shared")
        ag_qa_in = dram.tile([Q_LORA, SH], BF16)
        ag_qa_out = dram.tile([N_CORES * Q_LORA, SH], BF16, addr_space="Shared")
        H1, H2 = 3, 2            # A2A split: heads 0-2 fire early, 3-4 at end
        a2a_in1 = dram.tile([N_CORES * H1 * V_D, SH], BF16)
        a2a_out1 = dram.tile([N_CORES * H1 * V_D, SH], BF16)
        a2a_in2 = dram.tile([N_CORES * H2 * V_D, SH], BF16)
        a2a_out2 = dram.tile([N_CORES * H2 * V_D, SH], BF16)

        # ---- small constants ----
        ones128 = consts.tile([128, 1], BF16)
        nc.vector.memset(ones128[:], 1.0)
        eps_sb = consts.tile([1, 1], F32)
        nc.vector.memset(eps_sb[:], EPS)
        mask_sb = consts.tile([128, 4 * QB], BF16)
        nc.sync.dma_start(mask_sb[:], maskT.ap())
        cosF_sb = consts.tile([ROPE_D, S], F32)
        nc.sync.dma_start(cosF_sb[:], cosF.ap())
        sinF_sb = consts.tile([ROPE_D, S], F32)
        nc.sync.dma_start(sinF_sb[:], sinF.ap())
        qlnw_sb = consts.tile([128, MQ], F32)
        for m in range(MQ):
            nc.sync.dma_start(qlnw_sb[:, m:m + 1], qlnw.ap()[128 * m:128 * (m + 1), :])
        kvlnw_sb = consts.tile([128, MKV], F32)
        for m in range(MKV):
            nc.sync.dma_start(kvlnw_sb[:, m:m + 1], kvlnw.ap()[128 * m:128 * (m + 1), :])

        # ================= Phase A: a-projections on the S-shard =============
        with ExitStack() as phA:
            pa = phA.enter_context(tc.tile_pool(name="phA", bufs=1))
            pa_sq = phA.enter_context(tc.tile_pool(name="phA_sq", bufs=3))

            hT_sb = pa.tile([128, KO_H * SH], BF16)
            for ko in range(KO_H):
                nc.sync.dma_start(hT_sb[:, SH * ko:SH * (ko + 1)],
                                  hT.ap()[128 * ko:128 * (ko + 1), :])
            wqa_sb = pa.tile([128, KO_H * Q_LORA], BF16)
            for ko in range(KO_H):
                nc.sync.dma_start(wqa_sb[:, Q_LORA * ko:Q_LORA * (ko + 1)],
                                  wqa.ap()[128 * ko:128 * (ko + 1), :])
            wkva_sb = pa.tile([128, KO_H * (KV_LORA + 2 * ROPE_D)], BF16)
            for ko in range(KO_H):
                nc.sync.dma_start(wkva_sb[:, 320 * ko:320 * (ko + 1)],
                                  wkva.ap()[128 * ko:128 * (ko + 1), :])

            def aproj_norm(n_m, w_sb, wwidth, moff, lnw_sb, fan_in, dst, dst_row):
                """matmul (feature-major) + rmsnorm over features; writes bf16
                normalized output into dst rows [dst_row, dst_row+128*n_m)."""
                x_sb = pa.tile([128, n_m * SH], F32, name=f"x_sb_{fan_in}")
                ssum = ps_mid.tile([128, 512], F32, name=f"ssum_{fan_in}", tag="ps")
                for m in range(n_m):
                    x_ps = ps_mid.tile([128, 512], F32, name=f"xps_{fan_in}_{m}", tag="ps")
                    for ko in range(KO_H):
                        nc.tensor.matmul(
                            x_ps[:, :SH],
                            lhsT=w_sb[:, wwidth * ko + moff + 128 * m:
                                      wwidth * ko + moff + 128 * (m + 1)],
                            rhs=hT_sb[:, SH * ko:SH * (ko + 1)],
                            start=(ko == 0), stop=(ko == KO_H - 1))
                    xs = x_sb[:, SH * m:SH * (m + 1)]
                    nc.vector.tensor_copy(xs, x_ps[:, :SH])
                    sq = pa_sq.tile([128, SH], BF16, name="sq")
                    nc.vector.tensor_mul(sq[:], xs, xs)
                    nc.tensor.matmul(ssum[:1, :SH], lhsT=ones128[:], rhs=sq[:],
                                     start=(m == 0), stop=(m == n_m - 1))
                # rnorm = 1/sqrt(mean + eps), broadcast to all partitions
                srt = pa_sq.tile([1, SH], F32, name="srt")
                nc.scalar.activation(srt[:], ssum[:1, :SH], Sqrt,
                                     scale=1.0 / fan_in, bias=eps_sb[:])
                rcp = pa_sq.tile([1, SH], F32, name="rcp")
                nc.vector.reciprocal_approx_fast(rcp[:], srt[:])
                rbc = pa_sq.tile([128, SH], F32, name="rbc")
                nc.gpsimd.partition_broadcast(rbc[:], rcp[:])
                for m in range(n_m):
                    outm = pa_sq.tile([128, SH], BF16, name="outm")
                    nc.vector.scalar_tensor_tensor(
                        outm[:], x_sb[:, SH * m:SH * (m + 1)],
                        lnw_sb[:, m:m + 1], rbc[:], op0=mult, op1=mult)
                    nc.sync.dma_start(
                        dst[dst_row + 128 * m:dst_row + 128 * (m + 1), :], outm[:])

            # --- kv path first so its (small) AllGather can overlap qa work ---
            aproj_norm(MKV, wkva_sb, KV_LORA + 2 * ROPE_D, 0, kvlnw_sb, KV_LORA,
                       ag_kv_in, 0)

            # k_pe: cols 256:288 = pe, 288:320 = pre-rotated pe; rope, no norm
            kpe_ps = ps_mid.tile([128, 512], F32, tag="ps")
            for ko in range(KO_H):
                nc.tensor.matmul(
                    kpe_ps[:2 * ROPE_D, :SH],
                    lhsT=wkva_sb[:, 320 * ko + 256:320 * ko + 320],
                    rhs=hT_sb[:, SH * ko:SH * (ko + 1)],
                    start=(ko == 0), stop=(ko == KO_H - 1))
            cosS_sb = pa.tile([ROPE_D, SH], F32)
            nc.sync.dma_start(cosS_sb[:], cosS.ap())
            sinS_sb = pa.tile([ROPE_D, SH], F32)
            nc.sync.dma_start(sinS_sb[:], sinS.ap())
            t1 = pa_sq.tile([ROPE_D, SH], F32, name="t1")
            nc.vector.tensor_mul(t1[:], kpe_ps[:ROPE_D, :SH], cosS_sb[:])
            t2 = pa_sq.tile([ROPE_D, SH], F32, name="t2")
            nc.vector.tensor_mul(t2[:], kpe_ps[ROPE_D:2 * ROPE_D, :SH], sinS_sb[:])
            kpe_bf = pa_sq.tile([ROPE_D, SH], BF16, name="kpe_bf")
            nc.vector.tensor_add(kpe_bf[:], t1[:], t2[:])
            nc.sync.dma_start(ag_kv_in[KV_LORA:KVROW, :], kpe_bf[:])

            nc.gpsimd.collective_compute(
                "AllGather", mybir.AluOpType.bypass,
                replica_groups=[list(range(N_CORES))],
                ins=[ag_kv_in[:]], outs=[ag_kv_out[:]])

            aproj_norm(MQ, wqa_sb, Q_LORA, 0, qlnw_sb, Q_LORA, ag_qa_in, 0)

        nc.gpsimd.collective_compute(
            "AllGather", mybir.AluOpType.bypass,
            replica_groups=[list(range(N_CORES))],
            ins=[ag_qa_in[:]], outs=[ag_qa_out[:]])

        WO_PRE = 6
        wo_pre = top.enter_context(tc.tile_pool(name="wo_pre", bufs=1))
        wo_pre_tiles = []
        for ko in range(WO_PRE):
            wt = wo_pre.tile([128, HIDDEN], BF16, name=f"wop_{ko}")
            nc.sync.dma_start(wt[:], wo.ap()[128 * ko:128 * (ko + 1), :])
            wo_pre_tiles.append(wt)

        # ================= Phases B-D ========================================
        with ExitStack() as phB:
            pb = phB.enter_context(tc.tile_pool(name="phB", bufs=1))
            qh_pool = phB.enter_context(tc.tile_pool(name="qh", bufs=2))
            kf_pool = phB.enter_context(tc.tile_pool(name="kf", bufs=1))
            pT_pool = phB.enter_context(tc.tile_pool(name="pT", bufs=2))
            at_pool = phB.enter_context(tc.tile_pool(name="at", bufs=2))

            KVROW = KV_LORA + ROPE_D
            # -- kv-side: assemble gathered activations (available first) --
            kvN = pb.tile([128, MKV * S], BF16)
            for m in range(MKV):
                for c in range(N_CORES):
                    nc.sync.dma_start(
                        kvN[:, S * m + SH * c:S * m + SH * (c + 1)],
                        ag_kv_out[KVROW * c + 128 * m:KVROW * c + 128 * (m + 1), :])
            kpeT = pb.tile([ROPE_D, S], BF16)
            for c in range(N_CORES):
                nc.sync.dma_start(
                    kpeT[:, SH * c:SH * (c + 1)],
                    ag_kv_out[KVROW * c + KV_LORA:KVROW * (c + 1), :])

            # -- b-projection weights --
            wqb_sb = pb.tile([128, MQ * (HPC * 128)], BF16)
            for ko in range(MQ):
                nc.sync.dma_start(wqb_sb[:, 640 * ko:640 * (ko + 1)],
                                  wqb.ap()[128 * ko:128 * (ko + 1), :])
            wkvb_sb = pb.tile([128, MKV * (HPC * 128)], BF16)
            for ko in range(MKV):
                nc.sync.dma_start(wkvb_sb[:, 640 * ko:640 * (ko + 1)],
                                  wkvb.ap()[128 * ko:128 * (ko + 1), :])

            # -- v for all local heads, seq-major with a fused ones column --
            # layout: v_all[:, h*(16*65) + st*65 + 0:64] = v tile, col 64 = 1.0
            v_all = pb.tile([128, HPC * NKT * (V_D + 1)], BF16)
            v_view = v_all.rearrange("p (h st c) -> p h st c", h=HPC, st=NKT)
            nc.gpsimd.memset(v_view[:, :, :, V_D:V_D + 1], 1.0)
            for st in range(NKT):
                v_ps = ps_mid.tile([128, 512], F32, tag="ps")
                for ko in range(MKV):
                    rhs = wkvb_sb[:, 640 * ko:640 * (ko + 1)].rearrange(
                        "p (h d) -> p h d", d=128)[:, :, NOPE_D:NOPE_D + V_D]
                    nc.tensor.matmul(v_ps[:, :HPC * V_D],
                                     lhsT=kvN[:, S * ko + 128 * st:S * ko + 128 * (st + 1)],
                                     rhs=rhs,
                                     start=(ko == 0), stop=(ko == MKV - 1))
                for h in range(HPC):
                    nc.vector.tensor_copy(
                        v_all[:, h * (NKT * 65) + st * 65:h * (NKT * 65) + st * 65 + V_D],
                        v_ps[:, V_D * h:V_D * (h + 1)])

            # ---- k_full^T for all local heads (kv path only) ----
            kfs = []
            for h in range(HPC):
                kf = kf_pool.tile([QHD, S], BF16, name=f"kf{h}")
                for nb in range(NQB):
                    kn_ps = ps_mid.tile([128, 512], F32, name="kn_ps", tag="ps")
                    for ko in range(MKV):
                        nc.tensor.matmul(
                            kn_ps[:NOPE_D, :],
                            lhsT=wkvb_sb[:, 640 * ko + 128 * h:640 * ko + 128 * h + NOPE_D],
                            rhs=kvN[:, S * ko + QB * nb:S * ko + QB * (nb + 1)],
                            start=(ko == 0), stop=(ko == MKV - 1))
                    nc.any.tensor_copy(kf[:NOPE_D, QB * nb:QB * (nb + 1)],
                                       kn_ps[:NOPE_D, :])
                nc.vector.tensor_copy(kf[NOPE_D:QHD, :], kpeT[:])
                kfs.append(kf)

            # -- qa-side: assemble gathered activations --
            qaN = pb.tile([128, MQ * S], BF16)
            for m in range(MQ):
                for c in range(N_CORES):
                    nc.sync.dma_start(
                        qaN[:, S * m + SH * c:S * m + SH * (c + 1)],
                        ag_qa_out[Q_LORA * c + 128 * m:Q_LORA * c + 128 * (m + 1), :])

            for h in range(HPC):
                kf = kfs[h]
                # ---- q^T for head h: [96, S], rows 64:96 roped ----
                qh = qh_pool.tile([QHD, S], BF16, name="qh")
                for nb in range(NQB):
                    q_ps = ps_mid.tile([128, 512], F32, name="q_ps", tag="ps")
                    for ko in range(MQ):
                        nc.tensor.matmul(
                            q_ps[:],
                            lhsT=wqb_sb[:, 640 * ko + 128 * h:640 * ko + 128 * (h + 1)],
                            rhs=qaN[:, S * ko + QB * nb:S * ko + QB * (nb + 1)],
                            start=(ko == 0), stop=(ko == MQ - 1))
                    cs = slice(QB * nb, QB * (nb + 1))
                    nc.any.tensor_copy(qh[:NOPE_D, cs], q_ps[:NOPE_D, :])
                    t1 = at_pool.tile([ROPE_D, QB], F32, name="t1")
                    nc.vector.tensor_mul(t1[:], q_ps[NOPE_D:QHD, :], cosF_sb[:, cs])
                    t2 = at_pool.tile([ROPE_D, QB], F32, name="t2")
                    nc.vector.tensor_mul(t2[:], q_ps[QHD:QHD + ROPE_D, :], sinF_sb[:, cs])
                    nc.vector.tensor_add(qh[NOPE_D:QHD, cs], t1[:], t2[:])

                # ---- attention for head h ----
                for qb in range(NQB):
                    nk = 4 * (qb + 1)
                    pT = pT_pool.tile([128, NKT * QB], BF16, name="pT")
                    for g in range(nk // 2):
                        sc_ps = ps_big.tile([128, 1024], F32, name="sc_ps", tag="big")
                        for k2 in range(2):
                            kt = 2 * g + k2
                            nc.tensor.matmul(
                                sc_ps[:, 512 * k2:512 * (k2 + 1)],
                                lhsT=kf[:, KT * kt:KT * (kt + 1)],
                                rhs=qh[:, QB * qb:QB * (qb + 1)],
                                start=True, stop=True)
                        nc.scalar.activation(pT[:, 1024 * g:1024 * (g + 1)],
                                             sc_ps[:], Exp, scale=SCALE)
                    for i, kt in enumerate(range(4 * qb, 4 * qb + 4)):
                        nc.vector.tensor_mul(pT[:, QB * kt:QB * (kt + 1)],
                                             pT[:, QB * kt:QB * (kt + 1)],
                                             mask_sb[:, QB * i:QB * (i + 1)])
                    pv_ps = ps_mid.tile([128, 512], F32, name="pv_ps", tag="ps")
                    for kt in range(nk):
                        nc.tensor.matmul(
                            pv_ps[:V_D + 1, :],
                            lhsT=v_all[:, h * (NKT * 65) + 65 * kt:
                                       h * (NKT * 65) + 65 * kt + 65],
                            rhs=pT[:, QB * kt:QB * (kt + 1)],
                            start=(kt == 0), stop=(kt == nk - 1))
                    dn_sb = at_pool.tile([1, QB], F32, name="dn_sb")
                    nc.vector.tensor_copy(dn_sb[:], pv_ps[V_D:V_D + 1, :])
                    rcd = at_pool.tile([1, QB], F32, name="rcd")
                    nc.vector.reciprocal_approx_fast(rcd[:], dn_sb[:])
                    bc_sb = at_pool.tile([V_D, QB], F32, name="bc_sb")
                    nc.gpsimd.partition_broadcast(bc_sb[:], rcd[:])
                    attn_sb = at_pool.tile([V_D, QB], BF16, name="attn_sb")
                    nc.vector.tensor_mul(attn_sb[:], pv_ps[:V_D, :], bc_sb[:])
                    for half in range(2):
                        j2 = 2 * qb + half
                        if h < H1:
                            rowbase = (H1 * V_D) * j2 + V_D * h
                            dst = a2a_in1
                        else:
                            rowbase = (H2 * V_D) * j2 + V_D * (h - H1)
                            dst = a2a_in2
                        nc.sync.dma_start(
                            dst[rowbase:rowbase + V_D, :],
                            attn_sb[:, SH * half:SH * (half + 1)])

                if h == H1 - 1:
                    nc.gpsimd.collective_compute(
                        "AllToAll", mybir.AluOpType.bypass,
                        replica_groups=[list(range(N_CORES))],
                        ins=[a2a_in1[:]], outs=[a2a_out1[:]])

        # ================= AllToAll (heads 3-4) =============================
        nc.gpsimd.collective_compute(
            "AllToAll", mybir.AluOpType.bypass,
            replica_groups=[list(range(N_CORES))],
            ins=[a2a_in2[:]], outs=[a2a_out2[:]])

        # ================= Phase E: o-projection ============================
        with ExitStack() as phE:
            pe = phE.enter_context(tc.tile_pool(name="phE", bufs=1))
            attn_T = pe.tile([128, KO_H * SH], BF16)
            SPLIT = N_CORES * H1 * V_D // 128  # 12 k-tiles from A2A_1
            for ko in range(KO_H):
                if ko < SPLIT:
                    srcb = a2a_out1[128 * ko:128 * (ko + 1), :]
                else:
                    srcb = a2a_out2[128 * (ko - SPLIT):128 * (ko - SPLIT + 1), :]
                nc.sync.dma_start(attn_T[:, SH * ko:SH * (ko + 1)], srcb)
            wo_tiles = list(wo_pre_tiles)
            for ko in range(WO_PRE, KO_H):
                wt = pe.tile([128, HIDDEN], BF16, name=f"wo_{ko}")
                nc.sync.dma_start(wt[:], wo.ap()[128 * ko:128 * (ko + 1), :])
                wo_tiles.append(wt)
            for st in range(SH // 128):
                for no in range(HIDDEN // 512):
                    o_ps = ps_mid.tile([128, 512], F32, name="o_ps", tag="ps")
                    for ko in range(KO_H):
                        nc.tensor.matmul(
                            o_ps[:],
                            lhsT=attn_T[:, SH * ko + 128 * st:SH * ko + 128 * (st + 1)],
                            rhs=wo_tiles[ko][:, 512 * no:512 * (no + 1)],
                            start=(ko == 0), stop=(ko == KO_H - 1))
                    o_sb = pe.tile([128, 512], F32, name="o_sb", bufs=3)
                    nc.any.tensor_copy(o_sb[:], o_ps[:])
                    nc.sync.dma_start(
                        o_out.ap()[128 * st:128 * (st + 1), 512 * no:512 * (no + 1)],
                        o_sb[:])
    nc.compile()
    return nc


# =========================== host side ======================================

def _host_inputs(hidden_states, position_ids, w_qa, q_a_ln_w, w_qb, w_kva,
                 kv_a_ln_w, w_kvb, w_o):
    bf = ml_dtypes.bfloat16
    x = np.asarray(hidden_states, np.float32)[0]            # [S, HIDDEN]
    hT_full = np.ascontiguousarray(x.T).astype(bf)          # [HIDDEN, S]

    # rope cache gathered by position_ids (host-side prep; identity for arange)
    inv_freq = 1.0 / (ROPE_BASE ** (np.arange(0, ROPE_D, 2, dtype=np.float32) / ROPE_D))
    t = np.arange(S, dtype=np.float32)
    freqs = np.outer(t, inv_freq)
    emb = np.concatenate([freqs, freqs], axis=-1)           # [S, 32]
    cos = np.cos(emb).astype(np.float32)
    sin = np.sin(emb).astype(np.float32)
    pid = np.asarray(position_ids).reshape(-1).astype(np.int64)
    cosT = np.ascontiguousarray(cos[pid].T)                 # [32, S] f32
    sinT = np.ascontiguousarray(sin[pid].T)

    # causal mask diagonal tiles: m[i][k, q] = 1 if 128*i + k <= q (within 512)
    k_idx = np.arange(128)
    q_idx = np.arange(QB)
    masks = [((128 * i + k_idx[:, None]) <= q_idx[None, :]) for i in range(4)]
    maskT = np.concatenate(masks, axis=1).astype(bf)        # [128, 2048]

    def rot_cols(w):
        # rotate_half folded into weights: rot[:, :16] = -w[:, 16:], rot[:, 16:] = w[:, :16]
        h = ROPE_D // 2
        return np.concatenate([-w[..., h:], w[..., :h]], axis=-1)

    w_qa = np.asarray(w_qa, np.float32).astype(bf)
    w_kva = np.asarray(w_kva, np.float32)
    w_kva = np.concatenate([w_kva, rot_cols(w_kva[:, KV_LORA:])], axis=1).astype(bf)
    w_qb4 = np.asarray(w_qb, np.float32).reshape(Q_LORA, N_HEADS, QHD)
    w_qb4 = np.concatenate([w_qb4, rot_cols(w_qb4[:, :, NOPE_D:])], axis=2)
    w_kvb4 = np.asarray(w_kvb, np.float32).reshape(KV_LORA, N_HEADS, NOPE_D + V_D)
    w_o = np.asarray(w_o, np.float32)
    # permute rows to the split-A2A feature order: (rank, heads 0-2) then
    # (rank, heads 3-4), 64 v-dims per head
    perm_heads = ([5 * i + hh for i in range(N_CORES) for hh in range(3)] +
                  [5 * i + 3 + hh for i in range(N_CORES) for hh in range(2)])
    w_o = np.ascontiguousarray(
        w_o.reshape(N_HEADS, V_D, HIDDEN)[perm_heads].reshape(N_HEADS * V_D, HIDDEN)
    ).astype(bf)
    qln = np.asarray(q_a_ln_w, np.float32).reshape(Q_LORA, 1)
    kvln = np.asarray(kv_a_ln_w, np.float32).reshape(KV_LORA, 1)

    in_maps = []
    for c in range(N_CORES):
        hs = slice(SH * c, SH * (c + 1))
        heads = slice(HPC * c, HPC * (c + 1))
        in_maps.append({
            "hT": np.ascontiguousarray(hT_full[:, hs]),
            "wqa": w_qa,
            "wkva": w_kva,
            "qlnw": qln,
            "kvlnw": kvln,
            "wqb": np.ascontiguousarray(
                w_qb4[:, heads, :].reshape(Q_LORA, HPC * 128)).astype(bf),
            "wkvb": np.ascontiguousarray(
                w_kvb4[:, heads, :].reshape(KV_LORA, HPC * 128)).astype(bf),
            "wo": w_o,
            "cosS": np.ascontiguousarray(cosT[:, hs]),
            "sinS": np.ascontiguousarray(sinT[:, hs]),
            "cosF": cosT,
            "sinF": sinT,
            "maskT": maskT,
        })
    return in_maps


_CACHE = {}


def _get_runner():
    if "runner" not in _CACHE:
        from concourse.bass_utils import run_bass_kernel_spmd  # noqa: F401
        nc = build_nc()
        _CACHE["nc"] = nc
        _CACHE["runner"] = None
    return _CACHE["nc"]


def kernel(**inputs) -> np.ndarray:
    from concourse.bass_utils import run_bass_kernel_spmd
    nc = _get_runner()
    in_maps = _host_inputs(**inputs)
    res = run_bass_kernel_spmd(nc, in_maps, core_ids=list(range(N_CORES)))
    out = np.concatenate([res.results[c]["o_out"] for c in range(N_CORES)], axis=0)
    return out.reshape(1, S, HIDDEN).astype(np.float32)


# revision 17
# speedup vs baseline: 1.5684x; 1.0273x over previous
"""MiniCPM3 MLA attention (B=1, S=2048, 40 heads) on 8 Trainium2 NeuronCores.

Sharding: tensor-parallel over heads (5 heads/core) for q_b/kv_b/attention;
data-parallel over sequence for the low-rank a-projections (S/8 rows each,
then AllGather); o_proj row-parallel via AllToAll of per-head attention
outputs so each core computes full output channels for its S/8 sequence rows.

Device layout convention: activations are kept feature-major ("transposed",
features on SBUF partitions) so every matmul contraction runs over the
partition axis without any on-device transposes.

All matmuls run in bf16 (1 PE cycle/row) with fp32 PSUM accumulation, except
tiny fp32r helper matmuls (partition-axis sums / broadcasts). Softmax uses
multiplicative causal masking after exp (no max subtraction; logits are O(5)
so exp cannot overflow) and a fused ones-column in the PV matmul to produce
denominators.
"""
import sys
sys.path.insert(0, "/opt/trn_rl_repo")
from contextlib import ExitStack

import numpy as np
import ml_dtypes

import concourse.bass as bass
import concourse.mybir as mybir
import concourse.tile as tile
from concourse import bacc

# ---- problem dims (hardcoded per spec) ----
HIDDEN = 2560
N_HEADS = 40
Q_LORA = 768
KV_LORA = 256
ROPE_D = 32
NOPE_D = 64
V_D = 64
QHD = NOPE_D + ROPE_D            # 96
ROPE_BASE = 10000.0
EPS = 1e-6
SCALE = QHD ** -0.5

N_CORES = 8
S = 2048
SH = S // N_CORES                # 256 sequence rows per core
HPC = N_HEADS // N_CORES         # 5 heads per core
QB = 512                         # query block
NQB = S // QB                    # 4
KT = 128                         # key tile
NKT = S // KT                    # 16

F32 = mybir.dt.float32
F32R = mybir.dt.float32r
BF16 = mybir.dt.bfloat16

KO_H = HIDDEN // 128             # 20 k-tiles over hidden
MQ = Q_LORA // 128               # 6 m-tiles over q_lora
MKV = KV_LORA // 128             # 2 m-tiles over kv_lora
AGROW = Q_LORA + KV_LORA + ROPE_D  # 1056 rows in the all-gather payload


def r(ap):
    """bitcast an fp32 AP to fp32r for full-rate PE matmul."""
    return ap.bitcast(F32R)


def build_nc():
    nc = bacc.Bacc(trn_type="TRN2", target_bir_lowering=False, debug=False,
                   num_devices=N_CORES)

    # ---- per-core external inputs ----
    hT = nc.dram_tensor("hT", [HIDDEN, SH], BF16, kind="ExternalInput")
    wqa = nc.dram_tensor("wqa", [HIDDEN, Q_LORA], BF16, kind="ExternalInput")
    wkva = nc.dram_tensor("wkva", [HIDDEN, KV_LORA + 2 * ROPE_D], BF16, kind="ExternalInput")
    qlnw = nc.dram_tensor("qlnw", [Q_LORA, 1], F32, kind="ExternalInput")
    kvlnw = nc.dram_tensor("kvlnw", [KV_LORA, 1], F32, kind="ExternalInput")
    wqb = nc.dram_tensor("wqb", [Q_LORA, HPC * (QHD + ROPE_D)], BF16, kind="ExternalInput")
    wkvb = nc.dram_tensor("wkvb", [KV_LORA, HPC * (NOPE_D + V_D)], BF16, kind="ExternalInput")
    wo = nc.dram_tensor("wo", [N_HEADS * V_D, HIDDEN], BF16, kind="ExternalInput")
    cosS = nc.dram_tensor("cosS", [ROPE_D, SH], F32, kind="ExternalInput")
    sinS = nc.dram_tensor("sinS", [ROPE_D, SH], F32, kind="ExternalInput")
    cosF = nc.dram_tensor("cosF", [ROPE_D, S], F32, kind="ExternalInput")
    sinF = nc.dram_tensor("sinF", [ROPE_D, S], F32, kind="ExternalInput")
    maskT = nc.dram_tensor("maskT", [128, 4 * QB], BF16, kind="ExternalInput")

    # ---- per-core external output: o for this core's S-shard, [SH, HIDDEN] ----
    o_out = nc.dram_tensor("o_out", [SH, HIDDEN], F32, kind="ExternalOutput")

    Exp = mybir.ActivationFunctionType.Exp
    Sqrt = mybir.ActivationFunctionType.Sqrt
    mult = mybir.AluOpType.mult
    add = mybir.AluOpType.add

    with tile.TileContext(nc) as tc, ExitStack() as top:
        dram = top.enter_context(tc.tile_pool(name="dram", bufs=1, space="DRAM"))
        ps_mid = top.enter_context(tc.tile_pool(name="ps_mid", bufs=2, space="PSUM"))
        consts = top.enter_context(tc.tile_pool(name="consts", bufs=1))
        tmp = top.enter_context(tc.tile_pool(name="tmp", bufs=3))

        # ---- DRAM collective buffers ----
        KVROW = KV_LORA + ROPE_D  # 288
        ag_kv_in = dram.tile([KVROW, SH], BF16)
        ag_kv_out = dram.tile([N_CORES * KVROW, SH], BF16, addr_space="Shared")
        ag_qa_in = dram.tile([Q_LORA, SH], BF16)
        ag_qa_out = dram.tile([N_CORES * Q_LORA, SH], BF16, addr_space="Shared")
        H1, H2 = 3, 2            # A2A split: heads 0-2 fire early, 3-4 at end
        a2a_in1 = dram.tile([N_CORES * H1 * V_D, SH], BF16)
        a2a_out1 = dram.tile([N_CORES * H1 * V_D, SH], BF16)
        a2a_in2 = dram.tile([N_CORES * H2 * V_D, SH], BF16)
        a2a_out2 = dram.tile([N_CORES * H2 * V_D, SH], BF16)

        # ---- small constants ----
        ones128 = consts.tile([128, 1], BF16)
        nc.vector.memset(ones128[:], 1.0)
        eps_sb = consts.tile([1, 1], F32)
        nc.vector.memset(eps_sb[:], EPS)
        mask_sb = consts.tile([128, 4 * QB], BF16)
        nc.sync.dma_start(mask_sb[:], maskT.ap())
        cosF_sb = consts.tile([ROPE_D, S], F32)
        nc.sync.dma_start(cosF_sb[:], cosF.ap())
        sinF_sb = consts.tile([ROPE_D, S], F32)
        nc.sync.dma_start(sinF_sb[:], sinF.ap())
        qlnw_sb = consts.tile([128, MQ], F32)
        for m in range(MQ):
            nc.sync.dma_start(qlnw_sb[:, m:m + 1], qlnw.ap()[128 * m:128 * (m + 1), :])
        kvlnw_sb = consts.tile([128, MKV], F32)
        for m in range(MKV):
            nc.sync.dma_start(kvlnw_sb[:, m:m + 1], kvlnw.ap()[128 * m:128 * (m + 1), :])

        # ================= Phase A: a-projections on the S-shard =============
        with ExitStack() as phA:
            pa = phA.enter_context(tc.tile_pool(name="phA", bufs=1))
            pa_sq = phA.enter_context(tc.tile_pool(name="phA_sq", bufs=3))

            hT_sb = pa.tile([128, KO_H * SH], BF16)
            for ko in range(KO_H):
                nc.sync.dma_start(hT_sb[:, SH * ko:SH * (ko + 1)],
                                  hT.ap()[128 * ko:128 * (ko + 1), :])
            wqa_sb = pa.tile([128, KO_H * Q_LORA], BF16)
            for ko in range(KO_H):
                nc.sync.dma_start(wqa_sb[:, Q_LORA * ko:Q_LORA * (ko + 1)],
                                  wqa.ap()[128 * ko:128 * (ko + 1), :])
            wkva_sb = pa.tile([128, KO_H * (KV_LORA + 2 * ROPE_D)], BF16)
            for ko in range(KO_H):
                nc.sync.dma_start(wkva_sb[:, 320 * ko:320 * (ko + 1)],
                                  wkva.ap()[128 * ko:128 * (ko + 1), :])

            def aproj_norm(n_m, w_sb, wwidth, moff, lnw_sb, fan_in, dst, dst_row):
                """matmul (feature-major) + rmsnorm over features; writes bf16
                normalized output into dst rows [dst_row, dst_row+128*n_m)."""
                x_sb = pa.tile([128, n_m * SH], F32, name=f"x_sb_{fan_in}")
                ssum = ps_mid.tile([128, 512], F32, name=f"ssum_{fan_in}", tag="ps")
                for m in range(n_m):
                    x_ps = ps_mid.tile([128, 512], F32, name=f"xps_{fan_in}_{m}", tag="ps")
                    for ko in range(KO_H):
                        nc.tensor.matmul(
                            x_ps[:, :SH],
                            lhsT=w_sb[:, wwidth * ko + moff + 128 * m:
                                      wwidth * ko + moff + 128 * (m + 1)],
                            rhs=hT_sb[:, SH * ko:SH * (ko + 1)],
                            start=(ko == 0), stop=(ko == KO_H - 1))
                    xs = x_sb[:, SH * m:SH * (m + 1)]
                    nc.vector.tensor_copy(xs, x_ps[:, :SH])
                    sq = pa_sq.tile([128, SH], BF16, name="sq")
                    nc.vector.tensor_mul(sq[:], xs, xs)
                    nc.tensor.matmul(ssum[:1, :SH], lhsT=ones128[:], rhs=sq[:],
                                     start=(m == 0), stop=(m == n_m - 1))
                # rnorm = 1/sqrt(mean + eps), broadcast to all partitions
                srt = pa_sq.tile([1, SH], F32, name="srt")
                nc.scalar.activation(srt[:], ssum[:1, :SH], Sqrt,
                                     scale=1.0 / fan_in, bias=eps_sb[:])
                rcp = pa_sq.tile([1, SH], F32, name="rcp")
                nc.vector.reciprocal_approx_fast(rcp[:], srt[:])
                rbc = pa_sq.tile([128, SH], F32, name="rbc")
                nc.gpsimd.partition_broadcast(rbc[:], rcp[:])
                for m in range(n_m):
                    outm = pa_sq.tile([128, SH], BF16, name="outm")
                    nc.vector.scalar_tensor_tensor(
                        outm[:], x_sb[:, SH * m:SH * (m + 1)],
                        lnw_sb[:, m:m + 1], rbc[:], op0=mult, op1=mult)
                    nc.sync.dma_start(
                        dst[dst_row + 128 * m:dst_row + 128 * (m + 1), :], outm[:])

            # --- kv path first so its (small) AllGather can overlap qa work ---
            aproj_norm(MKV, wkva_sb, KV_LORA + 2 * ROPE_D, 0, kvlnw_sb, KV_LORA,
                       ag_kv_in, 0)

            # k_pe: cols 256:288 = pe, 288:320 = pre-rotated pe; rope, no norm
            kpe_ps = ps_mid.tile([128, 512], F32, tag="ps")
            for ko in range(KO_H):
                nc.tensor.matmul(
                    kpe_ps[:2 * ROPE_D, :SH],
                    lhsT=wkva_sb[:, 320 * ko + 256:320 * ko + 320],
                    rhs=hT_sb[:, SH * ko:SH * (ko + 1)],
                    start=(ko == 0), stop=(ko == KO_H - 1))
            cosS_sb = pa.tile([ROPE_D, SH], F32)
            nc.sync.dma_start(cosS_sb[:], cosS.ap())
            sinS_sb = pa.tile([ROPE_D, SH], F32)
            nc.sync.dma_start(sinS_sb[:], sinS.ap())
            t1 = pa_sq.tile([ROPE_D, SH], F32, name="t1")
            nc.vector.tensor_mul(t1[:], kpe_ps[:ROPE_D, :SH], cosS_sb[:])
            t2 = pa_sq.tile([ROPE_D, SH], F32, name="t2")
            nc.vector.tensor_mul(t2[:], kpe_ps[ROPE_D:2 * ROPE_D, :SH], sinS_sb[:])
            kpe_bf = pa_sq.tile([ROPE_D, SH], BF16, name="kpe_bf")
            nc.vector.tensor_add(kpe_bf[:], t1[:], t2[:])
            nc.sync.dma_start(ag_kv_in[KV_LORA:KVROW, :], kpe_bf[:])

            nc.gpsimd.collective_compute(
                "AllGather", mybir.AluOpType.bypass,
                replica_groups=[list(range(N_CORES))],
                ins=[ag_kv_in[:]], outs=[ag_kv_out[:]])

            aproj_norm(MQ, wqa_sb, Q_LORA, 0, qlnw_sb, Q_LORA, ag_qa_in, 0)

        nc.gpsimd.collective_compute(
            "AllGather", mybir.AluOpType.bypass,
            replica_groups=[list(range(N_CORES))],
            ins=[ag_qa_in[:]], outs=[ag_qa_out[:]])

        WO_PRE = 6
        wo_pre = top.enter_context(tc.tile_pool(name="wo_pre", bufs=1))
        wo_pre_tiles = []
        for ko in range(WO_PRE):
            wt = wo_pre.tile([128, HIDDEN], BF16, name=f"wop_{ko}")
            nc.sync.dma_start(wt[:], wo.ap()[128 * ko:128 * (ko + 1), :])
            wo_pre_tiles.append(wt)

        # ================= Phases B-D ========================================
        with ExitStack() as phB:
            pb = phB.enter_context(tc.tile_pool(name="phB", bufs=1))
            ps_big = phB.enter_context(tc.tile_pool(name="ps_big", bufs=3, space="PSUM"))
            qh_pool = phB.enter_context(tc.tile_pool(name="qh", bufs=2))
            kf_pool = phB.enter_context(tc.tile_pool(name="kf", bufs=1))
            pT_pool = phB.enter_context(tc.tile_pool(name="pT", bufs=2))
            at_pool = phB.enter_context(tc.tile_pool(name="at", bufs=2))

            KVROW = KV_LORA + ROPE_D
            # -- kv-side: assemble gathered activations (available first) --
            kvN = pb.tile([128, MKV * S], BF16)
            for m in range(MKV):
                for c in range(N_CORES):
                    nc.sync.dma_start(
                        kvN[:, S * m + SH * c:S * m + SH * (c + 1)],
                        ag_kv_out[KVROW * c + 128 * m:KVROW * c + 128 * (m + 1), :])
            kpeT = pb.tile([ROPE_D, S], BF16)
            for c in range(N_CORES):
                nc.sync.dma_start(
                    kpeT[:, SH * c:SH * (c + 1)],
                    ag_kv_out[KVROW * c + KV_LORA:KVROW * (c + 1), :])

            # -- b-projection weights --
            wqb_sb = pb.tile([128, MQ * (HPC * 128)], BF16)
            for ko in range(MQ):
                nc.sync.dma_start(wqb_sb[:, 640 * ko:640 * (ko + 1)],
                                  wqb.ap()[128 * ko:128 * (ko + 1), :])
            wkvb_sb = pb.tile([128, MKV * (HPC * 128)], BF16)
            for ko in range(MKV):
                nc.sync.dma_start(wkvb_sb[:, 640 * ko:640 * (ko + 1)],
                                  wkvb.ap()[128 * ko:128 * (ko + 1), :])

            # -- v for all local heads, seq-major with a fused ones column --
            # layout: v_all[:, h*(16*65) + st*65 + 0:64] = v tile, col 64 = 1.0
            v_all = pb.tile([128, HPC * NKT * (V_D + 1)], BF16)
            v_view = v_all.rearrange("p (h st c) -> p h st c", h=HPC, st=NKT)
            nc.gpsimd.memset(v_view[:, :, :, V_D:V_D + 1], 1.0)
            for st in range(NKT):
                v_ps = ps_mid.tile([128, 512], F32, tag="ps")
                for ko in range(MKV):
                    rhs = wkvb_sb[:, 640 * ko:640 * (ko + 1)].rearrange(
                        "p (h d) -> p h d", d=128)[:, :, NOPE_D:NOPE_D + V_D]
                    nc.tensor.matmul(v_ps[:, :HPC * V_D],
                                     lhsT=kvN[:, S * ko + 128 * st:S * ko + 128 * (st + 1)],
                                     rhs=rhs,
                                     start=(ko == 0), stop=(ko == MKV - 1))
                for h in range(HPC):
                    nc.vector.tensor_copy(
                        v_all[:, h * (NKT * 65) + st * 65:h * (NKT * 65) + st * 65 + V_D],
                        v_ps[:, V_D * h:V_D * (h + 1)])

            # ---- k_full^T for all local heads (kv path only) ----
            kfs = []
            for h in range(HPC):
                kf = kf_pool.tile([QHD, S], BF16, name=f"kf{h}")
                for nb in range(NQB):
                    kn_ps = ps_mid.tile([128, 512], F32, name="kn_ps", tag="ps")
                    for ko in range(MKV):
                        nc.tensor.matmul(
                            kn_ps[:NOPE_D, :],
                            lhsT=wkvb_sb[:, 640 * ko + 128 * h:640 * ko + 128 * h + NOPE_D],
                            rhs=kvN[:, S * ko + QB * nb:S * ko + QB * (nb + 1)],
                            start=(ko == 0), stop=(ko == MKV - 1))
                    nc.vector.tensor_copy(kf[:NOPE_D, QB * nb:QB * (nb + 1)],
                                          kn_ps[:NOPE_D, :])
                nc.vector.tensor_copy(kf[NOPE_D:QHD, :], kpeT[:])
                kfs.append(kf)

            # -- qa-side: assemble gathered activations --
            qaN = pb.tile([128, MQ * S], BF16)
            for m in range(MQ):
                for c in range(N_CORES):
                    nc.sync.dma_start(
                        qaN[:, S * m + SH * c:S * m + SH * (c + 1)],
                        ag_qa_out[Q_LORA * c + 128 * m:Q_LORA * c + 128 * (m + 1), :])

            for h in range(HPC):
                kf = kfs[h]
                # ---- q^T for head h: [96, S], rows 64:96 roped ----
                qh = qh_pool.tile([QHD, S], BF16, name="qh")
                for nb in range(NQB):
                    q_ps = ps_mid.tile([128, 512], F32, name="q_ps", tag="ps")
                    for ko in range(MQ):
                        nc.tensor.matmul(
                            q_ps[:],
                            lhsT=wqb_sb[:, 640 * ko + 128 * h:640 * ko + 128 * (h + 1)],
                            rhs=qaN[:, S * ko + QB * nb:S * ko + QB * (nb + 1)],
                            start=(ko == 0), stop=(ko == MQ - 1))
                    cs = slice(QB * nb, QB * (nb + 1))
                    nc.vector.tensor_copy(qh[:NOPE_D, cs], q_ps[:NOPE_D, :])
                    t1 = at_pool.tile([ROPE_D, QB], F32, name="t1")
                    nc.vector.tensor_mul(t1[:], q_ps[NOPE_D:QHD, :], cosF_sb[:, cs])
                    t2 = at_pool.tile([ROPE_D, QB], F32, name="t2")
                    nc.vector.tensor_mul(t2[:], q_ps[QHD:QHD + ROPE_D, :], sinF_sb[:, cs])
                    nc.vector.tensor_add(qh[NOPE_D:QHD, cs], t1[:], t2[:])

                # ---- attention for head h ----
                for qb in range(NQB):
                    nk = 4 * (qb + 1)
                    pT = pT_pool.tile([128, NKT * QB], BF16, name="pT")
                    for g in range(nk // 2):
                        sc_ps = ps_big.tile([128, 1024], F32, name="sc_ps", tag="big")
                        for k2 in range(2):
                            kt = 2 * g + k2
                            nc.tensor.matmul(
                                sc_ps[:, 512 * k2:512 * (k2 + 1)],
                                lhsT=kf[:, KT * kt:KT * (kt + 1)],
                                rhs=qh[:, QB * qb:QB * (qb + 1)],
                                start=True, stop=True)
                        nc.scalar.activation(pT[:, 1024 * g:1024 * (g + 1)],
                                             sc_ps[:], Exp, scale=SCALE)
                    for i, kt in enumerate(range(4 * qb, 4 * qb + 4)):
                        nc.vector.tensor_mul(pT[:, QB * kt:QB * (kt + 1)],
                                             pT[:, QB * kt:QB * (kt + 1)],
                                             mask_sb[:, QB * i:QB * (i + 1)])
                    pv_ps = ps_mid.tile([128, 512], F32, name="pv_ps", tag="ps")
                    for kt in range(nk):
                        nc.tensor.matmul(
                            pv_ps[:V_D + 1, :],
                            lhsT=v_all[:, h * (NKT * 65) + 65 * kt:
                                       h * (NKT * 65) + 65 * kt + 65],
                            rhs=pT[:, QB * kt:QB * (kt + 1)],
                            start=(kt == 0), stop=(kt == nk - 1))
                    dn_sb = at_pool.tile([1, QB], F32, name="dn_sb")
                    nc.vector.tensor_copy(dn_sb[:], pv_ps[V_D:V_D + 1, :])
                    rcd = at_pool.tile([1, QB], F32, name="rcd")
                    nc.vector.reciprocal_approx_fast(rcd[:], dn_sb[:])
                    bc_sb = at_pool.tile([V_D, QB], F32, name="bc_sb")
                    nc.gpsimd.partition_broadcast(bc_sb[:], rcd[:])
                    attn_sb = at_pool.tile([V_D, QB], BF16, name="attn_sb")
                    nc.vector.tensor_mul(attn_sb[:], pv_ps[:V_D, :], bc_sb[:])
                    for half in range(2):
                        j2 = 2 * qb + half
                        if h < H1:
                            rowbase = (H1 * V_D) * j2 + V_D * h
                            dst = a2a_in1
                        else:
                            rowbase = (H2 * V_D) * j2 + V_D * (h - H1)
                            dst = a2a_in2
                        nc.sync.dma_start(
                            dst[rowbase:rowbase + V_D, :],
                            attn_sb[:, SH * half:SH * (half + 1)])

                if h == H1 - 1:
                    nc.gpsimd.collective_compute(
                        "AllToAll", mybir.AluOpType.bypass,
                        replica_groups=[list(range(N_CORES))],
                        ins=[a2a_in1[:]], outs=[a2a_out1[:]])

        # ================= AllToAll (heads 3-4) =============================
        nc.gpsimd.collective_compute(
            "AllToAll", mybir.AluOpType.bypass,
            replica_groups=[list(range(N_CORES))],
            ins=[a2a_in2[:]], outs=[a2a_out2[:]])

        # ================= Phase E: o-projection ============================
        with ExitStack() as phE:
            pe = phE.enter_context(tc.tile_pool(name="phE", bufs=1))
            ps_o = phE.enter_context(tc.tile_pool(name="ps_o", bufs=1, space="PSUM"))
            attn_T = pe.tile([128, KO_H * SH], BF16)
            SPLIT = N_CORES * H1 * V_D // 128  # 12 k-tiles from A2A_1
            for ko in range(KO_H):
                if ko < SPLIT:
                    srcb = a2a_out1[128 * ko:128 * (ko + 1), :]
                else:
                    srcb = a2a_out2[128 * (ko - SPLIT):128 * (ko - SPLIT + 1), :]
                nc.sync.dma_start(attn_T[:, SH * ko:SH * (ko + 1)], srcb)
            wo_tiles = list(wo_pre_tiles)
            for ko in range(WO_PRE, KO_H):
                wt = pe.tile([128, HIDDEN], BF16, name=f"wo_{ko}")
                nc.sync.dma_start(wt[:], wo.ap()[128 * ko:128 * (ko + 1), :])
                wo_tiles.append(wt)
            NO = HIDDEN // 512
            for st in range(SH // 128):
                # ko-inner x no-inner: 5 psum groups live so each attn_T
                # stationary tile feeds 5 consecutive matmuls
                o_pss = [ps_o.tile([128, 512], F32, name=f"o_ps{st}_{no}",
                                    tag=f"ops{no}") for no in range(NO)]
                for ko in range(KO_H):
                    lt = attn_T[:, SH * ko + 128 * st:SH * ko + 128 * (st + 1)]
                    for no in range(NO):
                        nc.tensor.matmul(
                            o_pss[no][:],
                            lhsT=lt,
                            rhs=wo_tiles[ko][:, 512 * no:512 * (no + 1)],
                            start=(ko == 0), stop=(ko == KO_H - 1))
                for no in range(NO):
                    o_sb = pe.tile([128, 512], F32, name="o_sb", bufs=3)
                    nc.any.tensor_copy(o_sb[:], o_pss[no][:])
                    nc.sync.dma_start(
                        o_out.ap()[128 * st:128 * (st + 1), 512 * no:512 * (no + 1)],
                        o_sb[:])
    nc.compile()
    return nc


# =========================== host side ======================================

def _host_inputs(hidden_states, position_ids, w_qa, q_a_ln_w, w_qb, w_kva,
                 kv_a_ln_w, w_kvb, w_o):
    bf = ml_dtypes.bfloat16
    x = np.asarray(hidden_states, np.float32)[0]            # [S, HIDDEN]
    hT_full = np.ascontiguousarray(x.T).astype(bf)          # [HIDDEN, S]

    # rope cache gathered by position_ids (host-side prep; identity for arange)
    inv_freq = 1.0 / (ROPE_BASE ** (np.arange(0, ROPE_D, 2, dtype=np.float32) / ROPE_D))
    t = np.arange(S, dtype=np.float32)
    freqs = np.outer(t, inv_freq)
    emb = np.concatenate([freqs, freqs], axis=-1)           # [S, 32]
    cos = np.cos(emb).astype(np.float32)
    sin = np.sin(emb).astype(np.float32)
    pid = np.asarray(position_ids).reshape(-1).astype(np.int64)
    cosT = np.ascontiguousarray(cos[pid].T)                 # [32, S] f32
    sinT = np.ascontiguousarray(sin[pid].T)

    # causal mask diagonal tiles: m[i][k, q] = 1 if 128*i + k <= q (within 512)
    k_idx = np.arange(128)
    q_idx = np.arange(QB)
    masks = [((128 * i + k_idx[:, None]) <= q_idx[None, :]) for i in range(4)]
    maskT = np.concatenate(masks, axis=1).astype(bf)        # [128, 2048]

    def rot_cols(w):
        # rotate_half folded into weights: rot[:, :16] = -w[:, 16:], rot[:, 16:] = w[:, :16]
        h = ROPE_D // 2
        return np.concatenate([-w[..., h:], w[..., :h]], axis=-1)

    w_qa = np.asarray(w_qa, np.float32).astype(bf)
    w_kva = np.asarray(w_kva, np.float32)
    w_kva = np.concatenate([w_kva, rot_cols(w_kva[:, KV_LORA:])], axis=1).astype(bf)
    w_qb4 = np.asarray(w_qb, np.float32).reshape(Q_LORA, N_HEADS, QHD)
    w_qb4 = np.concatenate([w_qb4, rot_cols(w_qb4[:, :, NOPE_D:])], axis=2)
    w_kvb4 = np.asarray(w_kvb, np.float32).reshape(KV_LORA, N_HEADS, NOPE_D + V_D)
    w_o = np.asarray(w_o, np.float32)
    # permute rows to the split-A2A feature order: (rank, heads 0-2) then
    # (rank, heads 3-4), 64 v-dims per head
    perm_heads = ([5 * i + hh for i in range(N_CORES) for hh in range(3)] +
                  [5 * i + 3 + hh for i in range(N_CORES) for hh in range(2)])
    w_o = np.ascontiguousarray(
        w_o.reshape(N_HEADS, V_D, HIDDEN)[perm_heads].reshape(N_HEADS * V_D, HIDDEN)
    ).astype(bf)
    qln = np.asarray(q_a_ln_w, np.float32).reshape(Q_LORA, 1)
    kvln = np.asarray(kv_a_ln_w, np.float32).reshape(KV_LORA, 1)

    in_maps = []
    for c in range(N_CORES):
        hs = slice(SH * c, SH * (c + 1))
        heads = slice(HPC * c, HPC * (c + 1))
        in_maps.append({
            "hT": np.ascontiguousarray(hT_full[:, hs]),
            "wqa": w_qa,
            "wkva": w_kva,
            "qlnw": qln,
            "kvlnw": kvln,
            "wqb": np.ascontiguousarray(
                w_qb4[:, heads, :].reshape(Q_LORA, HPC * 128)).astype(bf),
            "wkvb": np.ascontiguousarray(
                w_kvb4[:, heads, :].reshape(KV_LORA, HPC * 128)).astype(bf),
            "wo": w_o,
            "cosS": np.ascontiguousarray(cosT[:, hs]),
            "sinS": np.ascontiguousarray(sinT[:, hs]),
            "cosF": cosT,
            "sinF": sinT,
            "maskT": maskT,
        })
    return in_maps


_CACHE = {}


def _get_runner():
    if "runner" not in _CACHE:
        from concourse.bass_utils import run_bass_kernel_spmd  # noqa: F401
        nc = build_nc()
        _CACHE["nc"] = nc
        _CACHE["runner"] = None
    return _CACHE["nc"]


def kernel(**inputs) -> np.ndarray:
    from concourse.bass_utils import run_bass_kernel_spmd
    nc = _get_runner()
    in_maps = _host_inputs(**inputs)
    res = run_bass_kernel_spmd(nc, in_maps, core_ids=list(range(N_CORES)))
    out = np.concatenate([res.results[c]["o_out"] for c in range(N_CORES)], axis=0)
    return out.reshape(1, S, HIDDEN).astype(np.float32)


# revision 18
# speedup vs baseline: 1.6278x; 1.0379x over previous
"""MiniCPM3 MLA attention (B=1, S=2048, 40 heads) on 8 Trainium2 NeuronCores.

Sharding: tensor-parallel over heads (5 heads/core) for q_b/kv_b/attention;
data-parallel over sequence for the low-rank a-projections (S/8 rows each,
then AllGather); o_proj row-parallel via AllToAll of per-head attention
outputs so each core computes full output channels for its S/8 sequence rows.

Device layout convention: activations are kept feature-major ("transposed",
features on SBUF partitions) so every matmul contraction runs over the
partition axis without any on-device transposes.

All matmuls run in bf16 (1 PE cycle/row) with fp32 PSUM accumulation, except
tiny fp32r helper matmuls (partition-axis sums / broadcasts). Softmax uses
multiplicative causal masking after exp (no max subtraction; logits are O(5)
so exp cannot overflow) and a fused ones-column in the PV matmul to produce
denominators.
"""
import sys
sys.path.insert(0, "/opt/trn_rl_repo")
from contextlib import ExitStack

import numpy as np
import ml_dtypes

import concourse.bass as bass
import concourse.mybir as mybir
import concourse.tile as tile
from concourse import bacc

# ---- problem dims (hardcoded per spec) ----
HIDDEN = 2560
N_HEADS = 40
Q_LORA = 768
KV_LORA = 256
ROPE_D = 32
NOPE_D = 64
V_D = 64
QHD = NOPE_D + ROPE_D            # 96
ROPE_BASE = 10000.0
EPS = 1e-6
SCALE = QHD ** -0.5

N_CORES = 8
S = 2048
SH = S // N_CORES                # 256 sequence rows per core
HPC = N_HEADS // N_CORES         # 5 heads per core
QB = 512                         # query block
NQB = S // QB                    # 4
KT = 128                         # key tile
NKT = S // KT                    # 16

F32 = mybir.dt.float32
F32R = mybir.dt.float32r
BF16 = mybir.dt.bfloat16

KO_H = HIDDEN // 128             # 20 k-tiles over hidden
MQ = Q_LORA // 128               # 6 m-tiles over q_lora
MKV = KV_LORA // 128             # 2 m-tiles over kv_lora
AGROW = Q_LORA + KV_LORA + ROPE_D  # 1056 rows in the all-gather payload


def r(ap):
    """bitcast an fp32 AP to fp32r for full-rate PE matmul."""
    return ap.bitcast(F32R)


def build_nc():
    nc = bacc.Bacc(trn_type="TRN2", target_bir_lowering=False, debug=False,
                   num_devices=N_CORES)

    # ---- per-core external inputs ----
    hT = nc.dram_tensor("hT", [HIDDEN, SH], BF16, kind="ExternalInput")
    wqa = nc.dram_tensor("wqa", [HIDDEN, Q_LORA], BF16, kind="ExternalInput")
    wkva = nc.dram_tensor("wkva", [HIDDEN, KV_LORA + 2 * ROPE_D], BF16, kind="ExternalInput")
    qlnw = nc.dram_tensor("qlnw", [Q_LORA, 1], F32, kind="ExternalInput")
    kvlnw = nc.dram_tensor("kvlnw", [KV_LORA, 1], F32, kind="ExternalInput")
    wqb = nc.dram_tensor("wqb", [Q_LORA, HPC * (QHD + ROPE_D)], BF16, kind="ExternalInput")
    wkvb = nc.dram_tensor("wkvb", [KV_LORA, HPC * (NOPE_D + V_D)], BF16, kind="ExternalInput")
    wo = nc.dram_tensor("wo", [N_HEADS * V_D, HIDDEN], BF16, kind="ExternalInput")
    cosS = nc.dram_tensor("cosS", [ROPE_D, SH], F32, kind="ExternalInput")
    sinS = nc.dram_tensor("sinS", [ROPE_D, SH], F32, kind="ExternalInput")
    cosF = nc.dram_tensor("cosF", [ROPE_D, S], F32, kind="ExternalInput")
    sinF = nc.dram_tensor("sinF", [ROPE_D, S], F32, kind="ExternalInput")
    maskT = nc.dram_tensor("maskT", [128, 4 * QB], BF16, kind="ExternalInput")

    # ---- per-core external output: o for this core's S-shard, [SH, HIDDEN] ----
    o_out = nc.dram_tensor("o_out", [SH, HIDDEN], F32, kind="ExternalOutput")

    Exp = mybir.ActivationFunctionType.Exp
    Sqrt = mybir.ActivationFunctionType.Sqrt
    mult = mybir.AluOpType.mult
    add = mybir.AluOpType.add

    with tile.TileContext(nc) as tc, ExitStack() as top:
        dram = top.enter_context(tc.tile_pool(name="dram", bufs=1, space="DRAM"))
        ps_mid = top.enter_context(tc.tile_pool(name="ps_mid", bufs=2, space="PSUM"))
        consts = top.enter_context(tc.tile_pool(name="consts", bufs=1))
        tmp = top.enter_context(tc.tile_pool(name="tmp", bufs=3))

        # ---- DRAM collective buffers ----
        KVROW = KV_LORA + ROPE_D  # 288
        ag_kv_in = dram.tile([KVROW, SH], BF16)
        ag_kv_out = dram.tile([N_CORES * KVROW, SH], BF16, addr_space="Shared")
        ag_qa_in = dram.tile([Q_LORA, SH], BF16)
        ag_qa_out = dram.tile([N_CORES * Q_LORA, SH], BF16, addr_space="Shared")
        H1, H2 = 3, 2            # A2A split: heads 0-2 fire early, 3-4 at end
        a2a_in1 = dram.tile([N_CORES * H1 * V_D, SH], BF16)
        a2a_out1 = dram.tile([N_CORES * H1 * V_D, SH], BF16)
        a2a_in2 = dram.tile([N_CORES * H2 * V_D, SH], BF16)
        a2a_out2 = dram.tile([N_CORES * H2 * V_D, SH], BF16)

        # ---- small constants ----
        ones128 = consts.tile([128, 1], BF16)
        nc.vector.memset(ones128[:], 1.0)
        eps_sb = consts.tile([1, 1], F32)
        nc.vector.memset(eps_sb[:], EPS)
        mask_sb = consts.tile([128, 4 * QB], BF16)
        nc.sync.dma_start(mask_sb[:], maskT.ap())
        cosF_sb = consts.tile([ROPE_D, S], F32)
        nc.sync.dma_start(cosF_sb[:], cosF.ap())
        sinF_sb = consts.tile([ROPE_D, S], F32)
        nc.sync.dma_start(sinF_sb[:], sinF.ap())
        qlnw_sb = consts.tile([128, MQ], F32)
        for m in range(MQ):
            nc.sync.dma_start(qlnw_sb[:, m:m + 1], qlnw.ap()[128 * m:128 * (m + 1), :])
        kvlnw_sb = consts.tile([128, MKV], F32)
        for m in range(MKV):
            nc.sync.dma_start(kvlnw_sb[:, m:m + 1], kvlnw.ap()[128 * m:128 * (m + 1), :])

        # ================= Phase A: a-projections on the S-shard =============
        with ExitStack() as phA:
            pa = phA.enter_context(tc.tile_pool(name="phA", bufs=1))
            pa_sq = phA.enter_context(tc.tile_pool(name="phA_sq", bufs=3))

            hT_sb = pa.tile([128, KO_H * SH], BF16)
            for ko in range(KO_H):
                nc.sync.dma_start(hT_sb[:, SH * ko:SH * (ko + 1)],
                                  hT.ap()[128 * ko:128 * (ko + 1), :])
            wkva_sb = pa.tile([128, KO_H * (KV_LORA + 2 * ROPE_D)], BF16)
            for ko in range(KO_H):
                nc.sync.dma_start(wkva_sb[:, 320 * ko:320 * (ko + 1)],
                                  wkva.ap()[128 * ko:128 * (ko + 1), :])
            wqa_sb = pa.tile([128, KO_H * Q_LORA], BF16)
            for ko in range(KO_H):
                nc.sync.dma_start(wqa_sb[:, Q_LORA * ko:Q_LORA * (ko + 1)],
                                  wqa.ap()[128 * ko:128 * (ko + 1), :])

            def aproj_norm(n_m, w_sb, wwidth, moff, lnw_sb, fan_in, dst, dst_row):
                """matmul (feature-major) + rmsnorm over features; writes bf16
                normalized output into dst rows [dst_row, dst_row+128*n_m)."""
                x_sb = pa.tile([128, n_m * SH], F32, name=f"x_sb_{fan_in}")
                ssum = ps_mid.tile([128, 512], F32, name=f"ssum_{fan_in}", tag="ps")
                for m in range(n_m):
                    x_ps = ps_mid.tile([128, 512], F32, name=f"xps_{fan_in}_{m}", tag="ps")
                    for ko in range(KO_H):
                        nc.tensor.matmul(
                            x_ps[:, :SH],
                            lhsT=w_sb[:, wwidth * ko + moff + 128 * m:
                                      wwidth * ko + moff + 128 * (m + 1)],
                            rhs=hT_sb[:, SH * ko:SH * (ko + 1)],
                            start=(ko == 0), stop=(ko == KO_H - 1))
                    xs = x_sb[:, SH * m:SH * (m + 1)]
                    nc.vector.tensor_copy(xs, x_ps[:, :SH])
                    sq = pa_sq.tile([128, SH], BF16, name="sq")
                    nc.vector.tensor_mul(sq[:], xs, xs)
                    nc.tensor.matmul(ssum[:1, :SH], lhsT=ones128[:], rhs=sq[:],
                                     start=(m == 0), stop=(m == n_m - 1))
                # rnorm = 1/sqrt(mean + eps), broadcast to all partitions
                srt = pa_sq.tile([1, SH], F32, name="srt")
                nc.scalar.activation(srt[:], ssum[:1, :SH], Sqrt,
                                     scale=1.0 / fan_in, bias=eps_sb[:])
                rcp = pa_sq.tile([1, SH], F32, name="rcp")
                nc.vector.reciprocal_approx_fast(rcp[:], srt[:])
                rbc = pa_sq.tile([128, SH], F32, name="rbc")
                nc.gpsimd.partition_broadcast(rbc[:], rcp[:])
                for m in range(n_m):
                    outm = pa_sq.tile([128, SH], BF16, name="outm")
                    nc.vector.scalar_tensor_tensor(
                        outm[:], x_sb[:, SH * m:SH * (m + 1)],
                        lnw_sb[:, m:m + 1], rbc[:], op0=mult, op1=mult)
                    nc.sync.dma_start(
                        dst[dst_row + 128 * m:dst_row + 128 * (m + 1), :], outm[:])

            # --- kv path first so its (small) AllGather can overlap qa work ---
            aproj_norm(MKV, wkva_sb, KV_LORA + 2 * ROPE_D, 0, kvlnw_sb, KV_LORA,
                       ag_kv_in, 0)

            # k_pe: cols 256:288 = pe, 288:320 = pre-rotated pe; rope, no norm
            kpe_ps = ps_mid.tile([128, 512], F32, tag="ps")
            for ko in range(KO_H):
                nc.tensor.matmul(
                    kpe_ps[:2 * ROPE_D, :SH],
                    lhsT=wkva_sb[:, 320 * ko + 256:320 * ko + 320],
                    rhs=hT_sb[:, SH * ko:SH * (ko + 1)],
                    start=(ko == 0), stop=(ko == KO_H - 1))
            cosS_sb = pa.tile([ROPE_D, SH], F32)
            nc.sync.dma_start(cosS_sb[:], cosS.ap())
            sinS_sb = pa.tile([ROPE_D, SH], F32)
            nc.sync.dma_start(sinS_sb[:], sinS.ap())
            t1 = pa_sq.tile([ROPE_D, SH], F32, name="t1")
            nc.vector.tensor_mul(t1[:], kpe_ps[:ROPE_D, :SH], cosS_sb[:])
            t2 = pa_sq.tile([ROPE_D, SH], F32, name="t2")
            nc.vector.tensor_mul(t2[:], kpe_ps[ROPE_D:2 * ROPE_D, :SH], sinS_sb[:])
            kpe_bf = pa_sq.tile([ROPE_D, SH], BF16, name="kpe_bf")
            nc.vector.tensor_add(kpe_bf[:], t1[:], t2[:])
            nc.sync.dma_start(ag_kv_in[KV_LORA:KVROW, :], kpe_bf[:])

            nc.gpsimd.collective_compute(
                "AllGather", mybir.AluOpType.bypass,
                replica_groups=[list(range(N_CORES))],
                ins=[ag_kv_in[:]], outs=[ag_kv_out[:]])

            aproj_norm(MQ, wqa_sb, Q_LORA, 0, qlnw_sb, Q_LORA, ag_qa_in, 0)

        nc.gpsimd.collective_compute(
            "AllGather", mybir.AluOpType.bypass,
            replica_groups=[list(range(N_CORES))],
            ins=[ag_qa_in[:]], outs=[ag_qa_out[:]])

        WO_PRE = 6
        wo_pre = top.enter_context(tc.tile_pool(name="wo_pre", bufs=1))
        wo_pre_tiles = []
        for ko in range(WO_PRE):
            wt = wo_pre.tile([128, HIDDEN], BF16, name=f"wop_{ko}")
            nc.sync.dma_start(wt[:], wo.ap()[128 * ko:128 * (ko + 1), :])
            wo_pre_tiles.append(wt)

        # ================= Phases B-D ========================================
        with ExitStack() as phB:
            pb = phB.enter_context(tc.tile_pool(name="phB", bufs=1))
            ps_big = phB.enter_context(tc.tile_pool(name="ps_big", bufs=3, space="PSUM"))
            qh_pool = phB.enter_context(tc.tile_pool(name="qh", bufs=2))
            kf_pool = phB.enter_context(tc.tile_pool(name="kf", bufs=1))
            pT_pool = phB.enter_context(tc.tile_pool(name="pT", bufs=2))
            at_pool = phB.enter_context(tc.tile_pool(name="at", bufs=2))

            KVROW = KV_LORA + ROPE_D
            # -- kv-side: assemble gathered activations (available first) --
            kvN = pb.tile([128, MKV * S], BF16)
            for m in range(MKV):
                for c in range(N_CORES):
                    nc.sync.dma_start(
                        kvN[:, S * m + SH * c:S * m + SH * (c + 1)],
                        ag_kv_out[KVROW * c + 128 * m:KVROW * c + 128 * (m + 1), :])
            kpeT = pb.tile([ROPE_D, S], BF16)
            for c in range(N_CORES):
                nc.sync.dma_start(
                    kpeT[:, SH * c:SH * (c + 1)],
                    ag_kv_out[KVROW * c + KV_LORA:KVROW * (c + 1), :])

            # -- b-projection weights --
            wqb_sb = pb.tile([128, MQ * (HPC * 128)], BF16)
            for ko in range(MQ):
                nc.sync.dma_start(wqb_sb[:, 640 * ko:640 * (ko + 1)],
                                  wqb.ap()[128 * ko:128 * (ko + 1), :])
            wkvb_sb = pb.tile([128, MKV * (HPC * 128)], BF16)
            for ko in range(MKV):
                nc.sync.dma_start(wkvb_sb[:, 640 * ko:640 * (ko + 1)],
                                  wkvb.ap()[128 * ko:128 * (ko + 1), :])

            # -- v for all local heads, seq-major with a fused ones column --
            # layout: v_all[:, h*(16*65) + st*65 + 0:64] = v tile, col 64 = 1.0
            v_all = pb.tile([128, HPC * NKT * (V_D + 1)], BF16)
            v_view = v_all.rearrange("p (h st c) -> p h st c", h=HPC, st=NKT)
            nc.gpsimd.memset(v_view[:, :, :, V_D:V_D + 1], 1.0)
            for st in range(NKT):
                v_ps = ps_mid.tile([128, 512], F32, tag="ps")
                for ko in range(MKV):
                    rhs = wkvb_sb[:, 640 * ko:640 * (ko + 1)].rearrange(
                        "p (h d) -> p h d", d=128)[:, :, NOPE_D:NOPE_D + V_D]
                    nc.tensor.matmul(v_ps[:, :HPC * V_D],
                                     lhsT=kvN[:, S * ko + 128 * st:S * ko + 128 * (st + 1)],
                                     rhs=rhs,
                                     start=(ko == 0), stop=(ko == MKV - 1))
                for h in range(HPC):
                    nc.vector.tensor_copy(
                        v_all[:, h * (NKT * 65) + st * 65:h * (NKT * 65) + st * 65 + V_D],
                        v_ps[:, V_D * h:V_D * (h + 1)])

            # ---- k_full^T for all local heads (kv path only) ----
            kfs = []
            for h in range(HPC):
                kf = kf_pool.tile([QHD, S], BF16, name=f"kf{h}")
                for nb in range(NQB):
                    kn_ps = ps_mid.tile([128, 512], F32, name="kn_ps", tag="ps")
                    for ko in range(MKV):
                        nc.tensor.matmul(
                            kn_ps[:NOPE_D, :],
                            lhsT=wkvb_sb[:, 640 * ko + 128 * h:640 * ko + 128 * h + NOPE_D],
                            rhs=kvN[:, S * ko + QB * nb:S * ko + QB * (nb + 1)],
                            start=(ko == 0), stop=(ko == MKV - 1))
                    nc.vector.tensor_copy(kf[:NOPE_D, QB * nb:QB * (nb + 1)],
                                          kn_ps[:NOPE_D, :])
                nc.vector.tensor_copy(kf[NOPE_D:QHD, :], kpeT[:])
                kfs.append(kf)

            # -- qa-side: assemble gathered activations --
            qaN = pb.tile([128, MQ * S], BF16)
            for m in range(MQ):
                for c in range(N_CORES):
                    nc.sync.dma_start(
                        qaN[:, S * m + SH * c:S * m + SH * (c + 1)],
                        ag_qa_out[Q_LORA * c + 128 * m:Q_LORA * c + 128 * (m + 1), :])

            for h in range(HPC):
                kf = kfs[h]
                # ---- q^T for head h: [96, S], rows 64:96 roped ----
                qh = qh_pool.tile([QHD, S], BF16, name="qh")
                for nb in range(NQB):
                    q_ps = ps_mid.tile([128, 512], F32, name="q_ps", tag="ps")
                    for ko in range(MQ):
                        nc.tensor.matmul(
                            q_ps[:],
                            lhsT=wqb_sb[:, 640 * ko + 128 * h:640 * ko + 128 * (h + 1)],
                            rhs=qaN[:, S * ko + QB * nb:S * ko + QB * (nb + 1)],
                            start=(ko == 0), stop=(ko == MQ - 1))
                    cs = slice(QB * nb, QB * (nb + 1))
                    nc.vector.tensor_copy(qh[:NOPE_D, cs], q_ps[:NOPE_D, :])
                    t1 = at_pool.tile([ROPE_D, QB], F32, name="t1")
                    nc.vector.tensor_mul(t1[:], q_ps[NOPE_D:QHD, :], cosF_sb[:, cs])
                    t2 = at_pool.tile([ROPE_D, QB], F32, name="t2")
                    nc.vector.tensor_mul(t2[:], q_ps[QHD:QHD + ROPE_D, :], sinF_sb[:, cs])
                    nc.vector.tensor_add(qh[NOPE_D:QHD, cs], t1[:], t2[:])

                # ---- attention for head h ----
                for qb in range(NQB):
                    nk = 4 * (qb + 1)
                    pT = pT_pool.tile([128, NKT * QB], BF16, name="pT")
                    for g in range(nk // 2):
                        sc_ps = ps_big.tile([128, 1024], F32, name="sc_ps", tag="big")
                        for k2 in range(2):
                            kt = 2 * g + k2
                            nc.tensor.matmul(
                                sc_ps[:, 512 * k2:512 * (k2 + 1)],
                                lhsT=kf[:, KT * kt:KT * (kt + 1)],
                                rhs=qh[:, QB * qb:QB * (qb + 1)],
                                start=True, stop=True)
                        nc.scalar.activation(pT[:, 1024 * g:1024 * (g + 1)],
                                             sc_ps[:], Exp, scale=SCALE)
                    for i, kt in enumerate(range(4 * qb, 4 * qb + 4)):
                        nc.vector.tensor_mul(pT[:, QB * kt:QB * (kt + 1)],
                                             pT[:, QB * kt:QB * (kt + 1)],
                                             mask_sb[:, QB * i:QB * (i + 1)])
                    pv_ps = ps_mid.tile([128, 512], F32, name="pv_ps", tag="ps")
                    for kt in range(nk):
                        nc.tensor.matmul(
                            pv_ps[:V_D + 1, :],
                            lhsT=v_all[:, h * (NKT * 65) + 65 * kt:
                                       h * (NKT * 65) + 65 * kt + 65],
                            rhs=pT[:, QB * kt:QB * (kt + 1)],
                            start=(kt == 0), stop=(kt == nk - 1))
                    dn_sb = at_pool.tile([1, QB], F32, name="dn_sb")
                    nc.vector.tensor_copy(dn_sb[:], pv_ps[V_D:V_D + 1, :])
                    rcd = at_pool.tile([1, QB], F32, name="rcd")
                    nc.vector.reciprocal_approx_fast(rcd[:], dn_sb[:])
                    bc_sb = at_pool.tile([V_D, QB], F32, name="bc_sb")
                    nc.gpsimd.partition_broadcast(bc_sb[:], rcd[:])
                    attn_sb = at_pool.tile([V_D, QB], BF16, name="attn_sb")
                    nc.vector.tensor_mul(attn_sb[:], pv_ps[:V_D, :], bc_sb[:])
                    for half in range(2):
                        j2 = 2 * qb + half
                        if h < H1:
                            rowbase = (H1 * V_D) * j2 + V_D * h
                            dst = a2a_in1
                        else:
                            rowbase = (H2 * V_D) * j2 + V_D * (h - H1)
                            dst = a2a_in2
                        nc.sync.dma_start(
                            dst[rowbase:rowbase + V_D, :],
                            attn_sb[:, SH * half:SH * (half + 1)])

                if h == H1 - 1:
                    nc.gpsimd.collective_compute(
                        "AllToAll", mybir.AluOpType.bypass,
                        replica_groups=[list(range(N_CORES))],
                        ins=[a2a_in1[:]], outs=[a2a_out1[:]])

        # ================= AllToAll (heads 3-4) =============================
        nc.gpsimd.collective_compute(
            "AllToAll", mybir.AluOpType.bypass,
            replica_groups=[list(range(N_CORES))],
            ins=[a2a_in2[:]], outs=[a2a_out2[:]])

        # ================= Phase E: o-projection ============================
        with ExitStack() as phE:
            pe = phE.enter_context(tc.tile_pool(name="phE", bufs=1))
            ps_o = phE.enter_context(tc.tile_pool(name="ps_o", bufs=1, space="PSUM"))
            attn_T = pe.tile([128, KO_H * SH], BF16)
            SPLIT = N_CORES * H1 * V_D // 128  # 12 k-tiles from A2A_1
            for ko in range(KO_H):
                if ko < SPLIT:
                    srcb = a2a_out1[128 * ko:128 * (ko + 1), :]
                else:
                    srcb = a2a_out2[128 * (ko - SPLIT):128 * (ko - SPLIT + 1), :]
                nc.sync.dma_start(attn_T[:, SH * ko:SH * (ko + 1)], srcb)
            wo_tiles = list(wo_pre_tiles)
            for ko in range(WO_PRE, KO_H):
                wt = pe.tile([128, HIDDEN], BF16, name=f"wo_{ko}")
                nc.sync.dma_start(wt[:], wo.ap()[128 * ko:128 * (ko + 1), :])
                wo_tiles.append(wt)
            NO = HIDDEN // 512
            for st in range(SH // 128):
                # ko-inner x no-inner: 5 psum groups live so each attn_T
                # stationary tile feeds 5 consecutive matmuls
                o_pss = [ps_o.tile([128, 512], F32, name=f"o_ps{st}_{no}",
                                    tag=f"ops{no}") for no in range(NO)]
                for ko in range(KO_H):
                    lt = attn_T[:, SH * ko + 128 * st:SH * ko + 128 * (st + 1)]
                    for no in range(NO):
                        nc.tensor.matmul(
                            o_pss[no][:],
                            lhsT=lt,
                            rhs=wo_tiles[ko][:, 512 * no:512 * (no + 1)],
                            start=(ko == 0), stop=(ko == KO_H - 1))
                for no in range(NO):
                    o_sb = pe.tile([128, 512], F32, name="o_sb", bufs=3)
                    nc.any.tensor_copy(o_sb[:], o_pss[no][:])
                    nc.sync.dma_start(
                        o_out.ap()[128 * st:128 * (st + 1), 512 * no:512 * (no + 1)],
                        o_sb[:])
    nc.compile()
    return nc


# =========================== host side ======================================

def _host_inputs(hidden_states, position_ids, w_qa, q_a_ln_w, w_qb, w_kva,
                 kv_a_ln_w, w_kvb, w_o):
    bf = ml_dtypes.bfloat16
    x = np.asarray(hidden_states, np.float32)[0]            # [S, HIDDEN]
    hT_full = np.ascontiguousarray(x.T).astype(bf)          # [HIDDEN, S]

    # rope cache gathered by position_ids (host-side prep; identity for arange)
    inv_freq = 1.0 / (ROPE_BASE ** (np.arange(0, ROPE_D, 2, dtype=np.float32) / ROPE_D))
    t = np.arange(S, dtype=np.float32)
    freqs = np.outer(t, inv_freq)
    emb = np.concatenate([freqs, freqs], axis=-1)           # [S, 32]
    cos = np.cos(emb).astype(np.float32)
    sin = np.sin(emb).astype(np.float32)
    pid = np.asarray(position_ids).reshape(-1).astype(np.int64)
    cosT = np.ascontiguousarray(cos[pid].T)                 # [32, S] f32
    sinT = np.ascontiguousarray(sin[pid].T)

    # causal mask diagonal tiles: m[i][k, q] = 1 if 128*i + k <= q (within 512)
    k_idx = np.arange(128)
    q_idx = np.arange(QB)
    masks = [((128 * i + k_idx[:, None]) <= q_idx[None, :]) for i in range(4)]
    maskT = np.concatenate(masks, axis=1).astype(bf)        # [128, 2048]

    def rot_cols(w):
        # rotate_half folded into weights: rot[:, :16] = -w[:, 16:], rot[:, 16:] = w[:, :16]
        h = ROPE_D // 2
        return np.concatenate([-w[..., h:], w[..., :h]], axis=-1)

    w_qa = np.asarray(w_qa, np.float32).astype(bf)
    w_kva = np.asarray(w_kva, np.float32)
    w_kva = np.concatenate([w_kva, rot_cols(w_kva[:, KV_LORA:])], axis=1).astype(bf)
    w_qb4 = np.asarray(w_qb, np.float32).reshape(Q_LORA, N_HEADS, QHD)
    w_qb4 = np.concatenate([w_qb4, rot_cols(w_qb4[:, :, NOPE_D:])], axis=2)
    w_kvb4 = np.asarray(w_kvb, np.float32).reshape(KV_LORA, N_HEADS, NOPE_D + V_D)
    w_o = np.asarray(w_o, np.float32)
    # permute rows to the split-A2A feature order: (rank, heads 0-2) then
    # (rank, heads 3-4), 64 v-dims per head
    perm_heads = ([5 * i + hh for i in range(N_CORES) for hh in range(3)] +
                  [5 * i + 3 + hh for i in range(N_CORES) for hh in range(2)])
    w_o = np.ascontiguousarray(
        w_o.reshape(N_HEADS, V_D, HIDDEN)[perm_heads].reshape(N_HEADS * V_D, HIDDEN)
    ).astype(bf)
    qln = np.asarray(q_a_ln_w, np.float32).reshape(Q_LORA, 1)
    kvln = np.asarray(kv_a_ln_w, np.float32).reshape(KV_LORA, 1)

    in_maps = []
    for c in range(N_CORES):
        hs = slice(SH * c, SH * (c + 1))
        heads = slice(HPC * c, HPC * (c + 1))
        in_maps.append({
            "hT": np.ascontiguousarray(hT_full[:, hs]),
            "wqa": w_qa,
            "wkva": w_kva,
            "qlnw": qln,
            "kvlnw": kvln,
            "wqb": np.ascontiguousarray(
                w_qb4[:, heads, :].reshape(Q_LORA, HPC * 128)).astype(bf),
            "wkvb": np.ascontiguousarray(
                w_kvb4[:, heads, :].reshape(KV_LORA, HPC * 128)).astype(bf),
            "wo": w_o,
            "cosS": np.ascontiguousarray(cosT[:, hs]),
            "sinS": np.ascontiguousarray(sinT[:, hs]),
            "cosF": cosT,
            "sinF": sinT,
            "maskT": maskT,
        })
    return in_maps


_CACHE = {}


def _get_runner():
    if "runner" not in _CACHE:
        from concourse.bass_utils import run_bass_kernel_spmd  # noqa: F401
        nc = build_nc()
        _CACHE["nc"] = nc
        _CACHE["runner"] = None
    return _CACHE["nc"]


def kernel(**inputs) -> np.ndarray:
    from concourse.bass_utils import run_bass_kernel_spmd
    nc = _get_runner()
    in_maps = _host_inputs(**inputs)
    res = run_bass_kernel_spmd(nc, in_maps, core_ids=list(range(N_CORES)))
    out = np.concatenate([res.results[c]["o_out"] for c in range(N_CORES)], axis=0)
    return out.reshape(1, S, HIDDEN).astype(np.float32)


# revision 23
# speedup vs baseline: 1.6696x; 1.0257x over previous
"""MiniCPM3 MLA attention (B=1, S=2048, 40 heads) on 8 Trainium2 NeuronCores.

Sharding: tensor-parallel over heads (5 heads/core) for q_b/kv_b/attention;
data-parallel over sequence for the low-rank a-projections (S/8 rows each,
then AllGather); o_proj row-parallel via AllToAll of per-head attention
outputs so each core computes full output channels for its S/8 sequence rows.

Device layout convention: activations are kept feature-major ("transposed",
features on SBUF partitions) so every matmul contraction runs over the
partition axis without any on-device transposes.

All matmuls run in bf16 (1 PE cycle/row) with fp32 PSUM accumulation.
rotate_half for RoPE is folded into pre-rotated weight copies on the host so
the device needs no partition-shuffles. Softmax uses multiplicative causal
masking after exp (no max subtraction; logits are O(5) so exp cannot
overflow), a fused ones-column in the PV matmul to produce denominators, and
a fast approximate reciprocal + gpsimd partition-broadcast to normalize.
The first AllToAll (heads 0-2) fires mid-attention to overlap with compute.
"""
import sys
sys.path.insert(0, "/opt/trn_rl_repo")
from contextlib import ExitStack

import numpy as np
import ml_dtypes

import concourse.bass as bass
import concourse.mybir as mybir
import concourse.tile as tile
from concourse import bacc

# ---- problem dims (hardcoded per spec) ----
HIDDEN = 2560
N_HEADS = 40
Q_LORA = 768
KV_LORA = 256
ROPE_D = 32
NOPE_D = 64
V_D = 64
QHD = NOPE_D + ROPE_D            # 96
ROPE_BASE = 10000.0
EPS = 1e-6
SCALE = QHD ** -0.5

N_CORES = 8
S = 2048
SH = S // N_CORES                # 256 sequence rows per core
HPC = N_HEADS // N_CORES         # 5 heads per core
QB = 512                         # query block
NQB = S // QB                    # 4
KT = 128                         # key tile
NKT = S // KT                    # 16

F32 = mybir.dt.float32
F32R = mybir.dt.float32r
BF16 = mybir.dt.bfloat16

KO_H = HIDDEN // 128             # 20 k-tiles over hidden
MQ = Q_LORA // 128               # 6 m-tiles over q_lora
MKV = KV_LORA // 128             # 2 m-tiles over kv_lora
AGROW = Q_LORA + KV_LORA + ROPE_D  # 1056 rows in the all-gather payload


def r(ap):
    """bitcast an fp32 AP to fp32r for full-rate PE matmul."""
    return ap.bitcast(F32R)


def build_nc():
    nc = bacc.Bacc(trn_type="TRN2", target_bir_lowering=False, debug=False,
                   num_devices=N_CORES)

    # ---- per-core external inputs ----
    hT = nc.dram_tensor("hT", [HIDDEN, SH], BF16, kind="ExternalInput")
    wqa = nc.dram_tensor("wqa", [HIDDEN, Q_LORA], BF16, kind="ExternalInput")
    wkva = nc.dram_tensor("wkva", [HIDDEN, KV_LORA + 2 * ROPE_D], BF16, kind="ExternalInput")
    qlnw = nc.dram_tensor("qlnw", [Q_LORA, 1], F32, kind="ExternalInput")
    kvlnw = nc.dram_tensor("kvlnw", [KV_LORA, 1], F32, kind="ExternalInput")
    wqb = nc.dram_tensor("wqb", [Q_LORA, HPC * (QHD + ROPE_D)], BF16, kind="ExternalInput")
    wkvb = nc.dram_tensor("wkvb", [KV_LORA, HPC * (NOPE_D + V_D)], BF16, kind="ExternalInput")
    wo = nc.dram_tensor("wo", [N_HEADS * V_D, HIDDEN], BF16, kind="ExternalInput")
    cosS = nc.dram_tensor("cosS", [ROPE_D, SH], F32, kind="ExternalInput")
    sinS = nc.dram_tensor("sinS", [ROPE_D, SH], F32, kind="ExternalInput")
    cosF = nc.dram_tensor("cosF", [ROPE_D, S], F32, kind="ExternalInput")
    sinF = nc.dram_tensor("sinF", [ROPE_D, S], F32, kind="ExternalInput")
    maskT = nc.dram_tensor("maskT", [128, 4 * QB], BF16, kind="ExternalInput")

    # ---- per-core external output: o for this core's S-shard, [SH, HIDDEN] ----
    o_out = nc.dram_tensor("o_out", [SH, HIDDEN], F32, kind="ExternalOutput")

    Exp = mybir.ActivationFunctionType.Exp
    Sqrt = mybir.ActivationFunctionType.Sqrt
    mult = mybir.AluOpType.mult
    add = mybir.AluOpType.add

    with tile.TileContext(nc) as tc, ExitStack() as top:
        dram = top.enter_context(tc.tile_pool(name="dram", bufs=1, space="DRAM"))
        ps_mid = top.enter_context(tc.tile_pool(name="ps_mid", bufs=2, space="PSUM"))
        consts = top.enter_context(tc.tile_pool(name="consts", bufs=1))
        tmp = top.enter_context(tc.tile_pool(name="tmp", bufs=3))

        # ---- DRAM collective buffers ----
        KVROW = KV_LORA + ROPE_D  # 288
        ag_kv_in = dram.tile([KVROW, SH], BF16)
        ag_kv_out = dram.tile([N_CORES * KVROW, SH], BF16, addr_space="Shared")
        ag_qa_in = dram.tile([Q_LORA, SH], BF16)
        ag_qa_out = dram.tile([N_CORES * Q_LORA, SH], BF16, addr_space="Shared")
        H1, H2 = 4, 1            # A2A split: heads 0-3 fire early, head 4 at end
        a2a_in1 = dram.tile([N_CORES * H1 * V_D, SH], BF16)
        a2a_out1 = dram.tile([N_CORES * H1 * V_D, SH], BF16)
        a2a_in2 = dram.tile([N_CORES * H2 * V_D, SH], BF16)
        a2a_out2 = dram.tile([N_CORES * H2 * V_D, SH], BF16)

        # ---- small constants ----
        ones128 = consts.tile([128, 1], BF16)
        nc.vector.memset(ones128[:], 1.0)
        eps_sb = consts.tile([1, 1], F32)
        nc.vector.memset(eps_sb[:], EPS)
        mask_sb = consts.tile([128, 4 * QB], BF16)
        nc.sync.dma_start(mask_sb[:], maskT.ap())
        cosF_sb = consts.tile([ROPE_D, S], F32)
        nc.sync.dma_start(cosF_sb[:], cosF.ap())
        sinF_sb = consts.tile([ROPE_D, S], F32)
        nc.sync.dma_start(sinF_sb[:], sinF.ap())
        qlnw_sb = consts.tile([128, MQ], F32)
        for m in range(MQ):
            nc.sync.dma_start(qlnw_sb[:, m:m + 1], qlnw.ap()[128 * m:128 * (m + 1), :])
        kvlnw_sb = consts.tile([128, MKV], F32)
        for m in range(MKV):
            nc.sync.dma_start(kvlnw_sb[:, m:m + 1], kvlnw.ap()[128 * m:128 * (m + 1), :])

        # ================= Phase A: a-projections on the S-shard =============
        with ExitStack() as phA:
            pa = phA.enter_context(tc.tile_pool(name="phA", bufs=1))
            pa_sq = phA.enter_context(tc.tile_pool(name="phA_sq", bufs=3))

            hT_sb = pa.tile([128, KO_H * SH], BF16)
            for ko in range(KO_H):
                nc.sync.dma_start(hT_sb[:, SH * ko:SH * (ko + 1)],
                                  hT.ap()[128 * ko:128 * (ko + 1), :])
            wkva_sb = pa.tile([128, KO_H * (KV_LORA + 2 * ROPE_D)], BF16)
            for ko in range(KO_H):
                nc.sync.dma_start(wkva_sb[:, 320 * ko:320 * (ko + 1)],
                                  wkva.ap()[128 * ko:128 * (ko + 1), :])
            wqa_sb = pa.tile([128, KO_H * Q_LORA], BF16)
            for ko in range(KO_H):
                nc.sync.dma_start(wqa_sb[:, Q_LORA * ko:Q_LORA * (ko + 1)],
                                  wqa.ap()[128 * ko:128 * (ko + 1), :])

            def aproj_norm(n_m, w_sb, wwidth, moff, lnw_sb, fan_in, dst, dst_row):
                """matmul (feature-major) + rmsnorm over features; writes bf16
                normalized output into dst rows [dst_row, dst_row+128*n_m)."""
                x_sb = pa.tile([128, n_m * SH], F32, name=f"x_sb_{fan_in}")
                ssum = ps_mid.tile([128, 512], F32, name=f"ssum_{fan_in}", tag="ps")
                for m in range(n_m):
                    x_ps = ps_mid.tile([128, 512], F32, name=f"xps_{fan_in}_{m}", tag="ps")
                    for ko in range(KO_H):
                        nc.tensor.matmul(
                            x_ps[:, :SH],
                            lhsT=w_sb[:, wwidth * ko + moff + 128 * m:
                                      wwidth * ko + moff + 128 * (m + 1)],
                            rhs=hT_sb[:, SH * ko:SH * (ko + 1)],
                            start=(ko == 0), stop=(ko == KO_H - 1))
                    xs = x_sb[:, SH * m:SH * (m + 1)]
                    nc.vector.tensor_copy(xs, x_ps[:, :SH])
                    sq = pa_sq.tile([128, SH], BF16, name="sq")
                    nc.vector.tensor_mul(sq[:], xs, xs)
                    nc.tensor.matmul(ssum[:1, :SH], lhsT=ones128[:], rhs=sq[:],
                                     start=(m == 0), stop=(m == n_m - 1))
                # rnorm = 1/sqrt(mean + eps), broadcast to all partitions
                srt = pa_sq.tile([1, SH], F32, name="srt")
                nc.scalar.activation(srt[:], ssum[:1, :SH], Sqrt,
                                     scale=1.0 / fan_in, bias=eps_sb[:])
                rcp = pa_sq.tile([1, SH], F32, name="rcp")
                nc.vector.reciprocal_approx_fast(rcp[:], srt[:])
                rbc = pa_sq.tile([128, SH], F32, name="rbc")
                nc.gpsimd.partition_broadcast(rbc[:], rcp[:])
                for m in range(n_m):
                    outm = pa_sq.tile([128, SH], BF16, name="outm")
                    nc.vector.scalar_tensor_tensor(
                        outm[:], x_sb[:, SH * m:SH * (m + 1)],
                        lnw_sb[:, m:m + 1], rbc[:], op0=mult, op1=mult)
                    nc.sync.dma_start(
                        dst[dst_row + 128 * m:dst_row + 128 * (m + 1), :], outm[:])

            # --- kv path first so its (small) AllGather can overlap qa work ---
            aproj_norm(MKV, wkva_sb, KV_LORA + 2 * ROPE_D, 0, kvlnw_sb, KV_LORA,
                       ag_kv_in, 0)

            # k_pe: cols 256:288 = pe, 288:320 = pre-rotated pe; rope, no norm
            kpe_ps = ps_mid.tile([128, 512], F32, tag="ps")
            for ko in range(KO_H):
                nc.tensor.matmul(
                    kpe_ps[:2 * ROPE_D, :SH],
                    lhsT=wkva_sb[:, 320 * ko + 256:320 * ko + 320],
                    rhs=hT_sb[:, SH * ko:SH * (ko + 1)],
                    start=(ko == 0), stop=(ko == KO_H - 1))
            cosS_sb = pa.tile([ROPE_D, SH], F32)
            nc.sync.dma_start(cosS_sb[:], cosS.ap())
            sinS_sb = pa.tile([ROPE_D, SH], F32)
            nc.sync.dma_start(sinS_sb[:], sinS.ap())
            t1 = pa_sq.tile([ROPE_D, SH], F32, name="t1")
            nc.vector.tensor_mul(t1[:], kpe_ps[:ROPE_D, :SH], cosS_sb[:])
            t2 = pa_sq.tile([ROPE_D, SH], F32, name="t2")
            nc.vector.tensor_mul(t2[:], kpe_ps[ROPE_D:2 * ROPE_D, :SH], sinS_sb[:])
            kpe_bf = pa_sq.tile([ROPE_D, SH], BF16, name="kpe_bf")
            nc.vector.tensor_add(kpe_bf[:], t1[:], t2[:])
            nc.sync.dma_start(ag_kv_in[KV_LORA:KVROW, :], kpe_bf[:])

            nc.gpsimd.collective_compute(
                "AllGather", mybir.AluOpType.bypass,
                replica_groups=[list(range(N_CORES))],
                ins=[ag_kv_in[:]], outs=[ag_kv_out[:]])

            aproj_norm(MQ, wqa_sb, Q_LORA, 0, qlnw_sb, Q_LORA, ag_qa_in, 0)

        nc.gpsimd.collective_compute(
            "AllGather", mybir.AluOpType.bypass,
            replica_groups=[list(range(N_CORES))],
            ins=[ag_qa_in[:]], outs=[ag_qa_out[:]])

        WO_PRE = 6
        wo_pre = top.enter_context(tc.tile_pool(name="wo_pre", bufs=1))
        wo_pre_tiles = []
        for ko in range(WO_PRE):
            wt = wo_pre.tile([128, HIDDEN], BF16, name=f"wop_{ko}")
            nc.sync.dma_start(wt[:], wo.ap()[128 * ko:128 * (ko + 1), :])
            wo_pre_tiles.append(wt)

        # ================= Phases B-D ========================================
        with ExitStack() as phB:
            pb = phB.enter_context(tc.tile_pool(name="phB", bufs=1))
            ps_big = phB.enter_context(tc.tile_pool(name="ps_big", bufs=3, space="PSUM"))
            qh_pool = phB.enter_context(tc.tile_pool(name="qh", bufs=2))
            kf_pool = phB.enter_context(tc.tile_pool(name="kf", bufs=1))
            pT_pool = phB.enter_context(tc.tile_pool(name="pT", bufs=2))
            at_pool = phB.enter_context(tc.tile_pool(name="at", bufs=2))

            KVROW = KV_LORA + ROPE_D
            # -- kv-side: assemble gathered activations (available first) --
            kvN = pb.tile([128, MKV * S], BF16)
            for m in range(MKV):
                for c in range(N_CORES):
                    nc.sync.dma_start(
                        kvN[:, S * m + SH * c:S * m + SH * (c + 1)],
                        ag_kv_out[KVROW * c + 128 * m:KVROW * c + 128 * (m + 1), :])
            kpeT = pb.tile([ROPE_D, S], BF16)
            for c in range(N_CORES):
                nc.sync.dma_start(
                    kpeT[:, SH * c:SH * (c + 1)],
                    ag_kv_out[KVROW * c + KV_LORA:KVROW * (c + 1), :])

            # -- b-projection weights --
            wqb_sb = pb.tile([128, MQ * (HPC * 128)], BF16)
            for ko in range(MQ):
                nc.sync.dma_start(wqb_sb[:, 640 * ko:640 * (ko + 1)],
                                  wqb.ap()[128 * ko:128 * (ko + 1), :])
            wkvb_sb = pb.tile([128, MKV * (HPC * 128)], BF16)
            for ko in range(MKV):
                nc.sync.dma_start(wkvb_sb[:, 640 * ko:640 * (ko + 1)],
                                  wkvb.ap()[128 * ko:128 * (ko + 1), :])

            # -- v for all local heads, seq-major with a fused ones column --
            # layout: v_all[:, h*(16*65) + st*65 + 0:64] = v tile, col 64 = 1.0
            v_all = pb.tile([128, HPC * NKT * (V_D + 1)], BF16)
            v_view = v_all.rearrange("p (h st c) -> p h st c", h=HPC, st=NKT)
            nc.gpsimd.memset(v_view[:, :, :, V_D:V_D + 1], 1.0)
            for st in range(NKT):
                v_ps = ps_mid.tile([128, 512], F32, tag="ps")
                for ko in range(MKV):
                    rhs = wkvb_sb[:, 640 * ko:640 * (ko + 1)].rearrange(
                        "p (h d) -> p h d", d=128)[:, :, NOPE_D:NOPE_D + V_D]
                    nc.tensor.matmul(v_ps[:, :HPC * V_D],
                                     lhsT=kvN[:, S * ko + 128 * st:S * ko + 128 * (st + 1)],
                                     rhs=rhs,
                                     start=(ko == 0), stop=(ko == MKV - 1))
                for h in range(HPC):
                    nc.vector.tensor_copy(
                        v_all[:, h * (NKT * 65) + st * 65:h * (NKT * 65) + st * 65 + V_D],
                        v_ps[:, V_D * h:V_D * (h + 1)])

            # ---- k_full^T for all local heads (kv path only) ----
            kfs = []
            for h in range(HPC):
                kf = kf_pool.tile([QHD, S], BF16, name=f"kf{h}")
                for nb in range(NQB):
                    kn_ps = ps_mid.tile([128, 512], F32, name="kn_ps", tag="ps")
                    for ko in range(MKV):
                        nc.tensor.matmul(
                            kn_ps[:NOPE_D, :],
                            lhsT=wkvb_sb[:, 640 * ko + 128 * h:640 * ko + 128 * h + NOPE_D],
                            rhs=kvN[:, S * ko + QB * nb:S * ko + QB * (nb + 1)],
                            start=(ko == 0), stop=(ko == MKV - 1))
                    nc.vector.tensor_copy(kf[:NOPE_D, QB * nb:QB * (nb + 1)],
                                          kn_ps[:NOPE_D, :])
                nc.vector.tensor_copy(kf[NOPE_D:QHD, :], kpeT[:])
                kfs.append(kf)

            # -- qa-side: assemble gathered activations --
            qaN = pb.tile([128, MQ * S], BF16)
            for m in range(MQ):
                for c in range(N_CORES):
                    nc.sync.dma_start(
                        qaN[:, S * m + SH * c:S * m + SH * (c + 1)],
                        ag_qa_out[Q_LORA * c + 128 * m:Q_LORA * c + 128 * (m + 1), :])

            for h in range(HPC):
                kf = kfs[h]
                # ---- q^T for head h: [96, S], rows 64:96 roped ----
                qh = qh_pool.tile([QHD, S], BF16, name="qh")
                for nb in range(NQB):
                    q_ps = ps_mid.tile([128, 512], F32, name="q_ps", tag="ps")
                    for ko in range(MQ):
                        nc.tensor.matmul(
                            q_ps[:],
                            lhsT=wqb_sb[:, 640 * ko + 128 * h:640 * ko + 128 * (h + 1)],
                            rhs=qaN[:, S * ko + QB * nb:S * ko + QB * (nb + 1)],
                            start=(ko == 0), stop=(ko == MQ - 1))
                    cs = slice(QB * nb, QB * (nb + 1))
                    nc.vector.tensor_copy(qh[:NOPE_D, cs], q_ps[:NOPE_D, :])
                    t1 = at_pool.tile([ROPE_D, QB], F32, name="t1")
                    nc.vector.tensor_mul(t1[:], q_ps[NOPE_D:QHD, :], cosF_sb[:, cs])
                    t2 = at_pool.tile([ROPE_D, QB], F32, name="t2")
                    nc.vector.tensor_mul(t2[:], q_ps[QHD:QHD + ROPE_D, :], sinF_sb[:, cs])
                    nc.vector.tensor_add(qh[NOPE_D:QHD, cs], t1[:], t2[:])

                # ---- attention for head h ----
                for qb in range(NQB):
                    nk = 4 * (qb + 1)
                    pT = pT_pool.tile([128, NKT * QB], BF16, name="pT")
                    for g in range(nk // 2):
                        sc_ps = ps_big.tile([128, 1024], F32, name="sc_ps", tag="big")
                        for k2 in range(2):
                            kt = 2 * g + k2
                            nc.tensor.matmul(
                                sc_ps[:, 512 * k2:512 * (k2 + 1)],
                                lhsT=kf[:, KT * kt:KT * (kt + 1)],
                                rhs=qh[:, QB * qb:QB * (qb + 1)],
                                start=True, stop=True)
                        nc.scalar.activation(pT[:, 1024 * g:1024 * (g + 1)],
                                             sc_ps[:], Exp, scale=SCALE)
                    for i, kt in enumerate(range(4 * qb, 4 * qb + 4)):
                        nc.vector.tensor_mul(pT[:, QB * kt:QB * (kt + 1)],
                                             pT[:, QB * kt:QB * (kt + 1)],
                                             mask_sb[:, QB * i:QB * (i + 1)])
                    pv_ps = ps_mid.tile([128, 512], F32, name="pv_ps", tag="ps")
                    for kt in range(nk):
                        nc.tensor.matmul(
                            pv_ps[:V_D + 1, :],
                            lhsT=v_all[:, h * (NKT * 65) + 65 * kt:
                                       h * (NKT * 65) + 65 * kt + 65],
                            rhs=pT[:, QB * kt:QB * (kt + 1)],
                            start=(kt == 0), stop=(kt == nk - 1))
                    dn_sb = at_pool.tile([1, QB], F32, name="dn_sb")
                    nc.vector.tensor_copy(dn_sb[:], pv_ps[V_D:V_D + 1, :])
                    rcd = at_pool.tile([1, QB], F32, name="rcd")
                    nc.vector.reciprocal_approx_fast(rcd[:], dn_sb[:])
                    bc_sb = at_pool.tile([V_D, QB], F32, name="bc_sb")
                    nc.gpsimd.partition_broadcast(bc_sb[:], rcd[:])
                    attn_sb = at_pool.tile([V_D, QB], BF16, name="attn_sb")
                    nc.vector.tensor_mul(attn_sb[:], pv_ps[:V_D, :], bc_sb[:])
                    for half in range(2):
                        j2 = 2 * qb + half
                        if h < H1:
                            rowbase = (H1 * V_D) * j2 + V_D * h
                            dst = a2a_in1
                        else:
                            rowbase = (H2 * V_D) * j2 + V_D * (h - H1)
                            dst = a2a_in2
                        nc.sync.dma_start(
                            dst[rowbase:rowbase + V_D, :],
                            attn_sb[:, SH * half:SH * (half + 1)])

                if h == H1 - 1:
                    nc.gpsimd.collective_compute(
                        "AllToAll", mybir.AluOpType.bypass,
                        replica_groups=[list(range(N_CORES))],
                        ins=[a2a_in1[:]], outs=[a2a_out1[:]])

        # ================= AllToAll (heads 3-4) =============================
        nc.gpsimd.collective_compute(
            "AllToAll", mybir.AluOpType.bypass,
            replica_groups=[list(range(N_CORES))],
            ins=[a2a_in2[:]], outs=[a2a_out2[:]])

        # ================= Phase E: o-projection ============================
        with ExitStack() as phE:
            pe = phE.enter_context(tc.tile_pool(name="phE", bufs=1))
            ps_o = phE.enter_context(tc.tile_pool(name="ps_o", bufs=1, space="PSUM"))
            attn_T = pe.tile([128, KO_H * SH], BF16)
            SPLIT = N_CORES * H1 * V_D // 128  # 12 k-tiles from A2A_1
            for ko in range(KO_H):
                if ko < SPLIT:
                    srcb = a2a_out1[128 * ko:128 * (ko + 1), :]
                else:
                    srcb = a2a_out2[128 * (ko - SPLIT):128 * (ko - SPLIT + 1), :]
                nc.sync.dma_start(attn_T[:, SH * ko:SH * (ko + 1)], srcb)
            wo_tiles = list(wo_pre_tiles)
            for ko in range(WO_PRE, KO_H):
                wt = pe.tile([128, HIDDEN], BF16, name=f"wo_{ko}")
                nc.sync.dma_start(wt[:], wo.ap()[128 * ko:128 * (ko + 1), :])
                wo_tiles.append(wt)
            NO = HIDDEN // 512
            for st in range(SH // 128):
                # ko-inner x no-inner: 5 psum groups live so each attn_T
                # stationary tile feeds 5 consecutive matmuls
                o_pss = [ps_o.tile([128, 512], F32, name=f"o_ps{st}_{no}",
                                    tag="ops", bufs=6) for no in range(NO)]
                for ko in range(KO_H):
                    lt = attn_T[:, SH * ko + 128 * st:SH * ko + 128 * (st + 1)]
                    for no in range(NO):
                        nc.tensor.matmul(
                            o_pss[no][:],
                            lhsT=lt,
                            rhs=wo_tiles[ko][:, 512 * no:512 * (no + 1)],
                            start=(ko == 0), stop=(ko == KO_H - 1))
                for no in range(NO):
                    o_sb = pe.tile([128, 512], F32, name="o_sb", bufs=3)
                    nc.any.tensor_copy(o_sb[:], o_pss[no][:])
                    nc.sync.dma_start(
                        o_out.ap()[128 * st:128 * (st + 1), 512 * no:512 * (no + 1)],
                        o_sb[:])
    nc.compile()
    return nc


# =========================== host side ======================================

def _host_inputs(hidden_states, position_ids, w_qa, q_a_ln_w, w_qb, w_kva,
                 kv_a_ln_w, w_kvb, w_o):
    bf = ml_dtypes.bfloat16
    x = np.asarray(hidden_states, np.float32)[0]            # [S, HIDDEN]
    hT_full = np.ascontiguousarray(x.T).astype(bf)          # [HIDDEN, S]

    # rope cache gathered by position_ids (host-side prep; identity for arange)
    inv_freq = 1.0 / (ROPE_BASE ** (np.arange(0, ROPE_D, 2, dtype=np.float32) / ROPE_D))
    t = np.arange(S, dtype=np.float32)
    freqs = np.outer(t, inv_freq)
    emb = np.concatenate([freqs, freqs], axis=-1)           # [S, 32]
    cos = np.cos(emb).astype(np.float32)
    sin = np.sin(emb).astype(np.float32)
    pid = np.asarray(position_ids).reshape(-1).astype(np.int64)
    cosT = np.ascontiguousarray(cos[pid].T)                 # [32, S] f32
    sinT = np.ascontiguousarray(sin[pid].T)

    # causal mask diagonal tiles: m[i][k, q] = 1 if 128*i + k <= q (within 512)
    k_idx = np.arange(128)
    q_idx = np.arange(QB)
    masks = [((128 * i + k_idx[:, None]) <= q_idx[None, :]) for i in range(4)]
    maskT = np.concatenate(masks, axis=1).astype(bf)        # [128, 2048]

    def rot_cols(w):
        # rotate_half folded into weights: rot[:, :16] = -w[:, 16:], rot[:, 16:] = w[:, :16]
        h = ROPE_D // 2
        return np.concatenate([-w[..., h:], w[..., :h]], axis=-1)

    w_qa = np.asarray(w_qa, np.float32).astype(bf)
    w_kva = np.asarray(w_kva, np.float32)
    w_kva = np.concatenate([w_kva, rot_cols(w_kva[:, KV_LORA:])], axis=1).astype(bf)
    w_qb4 = np.asarray(w_qb, np.float32).reshape(Q_LORA, N_HEADS, QHD)
    w_qb4 = np.concatenate([w_qb4, rot_cols(w_qb4[:, :, NOPE_D:])], axis=2)
    w_kvb4 = np.asarray(w_kvb, np.float32).reshape(KV_LORA, N_HEADS, NOPE_D + V_D)
    w_o = np.asarray(w_o, np.float32)
    # permute rows to the split-A2A feature order: (rank, heads 0-2) then
    # (rank, heads 3-4), 64 v-dims per head
    perm_heads = ([5 * i + hh for i in range(N_CORES) for hh in range(4)] +
                  [5 * i + 4 for i in range(N_CORES)])
    w_o = np.ascontiguousarray(
        w_o.reshape(N_HEADS, V_D, HIDDEN)[perm_heads].reshape(N_HEADS * V_D, HIDDEN)
    ).astype(bf)
    qln = np.asarray(q_a_ln_w, np.float32).reshape(Q_LORA, 1)
    kvln = np.asarray(kv_a_ln_w, np.float32).reshape(KV_LORA, 1)

    in_maps = []
    for c in range(N_CORES):
        hs = slice(SH * c, SH * (c + 1))
        heads = slice(HPC * c, HPC * (c + 1))
        in_maps.append({
            "hT": np.ascontiguousarray(hT_full[:, hs]),
            "wqa": w_qa,
            "wkva": w_kva,
            "qlnw": qln,
            "kvlnw": kvln,
            "wqb": np.ascontiguousarray(
                w_qb4[:, heads, :].reshape(Q_LORA, HPC * 128)).astype(bf),
            "wkvb": np.ascontiguousarray(
                w_kvb4[:, heads, :].reshape(KV_LORA, HPC * 128)).astype(bf),
            "wo": w_o,
            "cosS": np.ascontiguousarray(cosT[:, hs]),
            "sinS": np.ascontiguousarray(sinT[:, hs]),
            "cosF": cosT,
            "sinF": sinT,
            "maskT": maskT,
        })
    return in_maps


_CACHE = {}


def _get_runner():
    if "runner" not in _CACHE:
        from concourse.bass_utils import run_bass_kernel_spmd  # noqa: F401
        nc = build_nc()
        _CACHE["nc"] = nc
        _CACHE["runner"] = None
    return _CACHE["nc"]


def kernel(**inputs) -> np.ndarray:
    from concourse.bass_utils import run_bass_kernel_spmd
    nc = _get_runner()
    in_maps = _host_inputs(**inputs)
    res = run_bass_kernel_spmd(nc, in_maps, core_ids=list(range(N_CORES)))
    out = np.concatenate([res.results[c]["o_out"] for c in range(N_CORES)], axis=0)
    return out.reshape(1, S, HIDDEN).astype(np.float32)
